# revision 53
# baseline (speedup 1.0000x reference)
"""Trainium2 Bass kernel for nn_AdaptiveAttention (sparse attention, B=4 S=1024 HID=1024 H=16).

Sharding (8 cores): core c = (batch b=c//2) x (head-group g=c%2, 8 heads / 512 hid cols).

v2 design (cost-model driven; ~130.7us vs 151.9us v1 baseline):
- All DRAM inputs host-pre-tiled into exact SBUF layouts so every DMA is a
  contiguous >=1KB-run burst (full-rate in the DMA model; elem runs >=512B).
- Q^T/K^T = W x x^T with temperature/sqrt(D) folded into the Q eviction
  (DVE tensor_scalar).
- Attention runs in 16 quarter-windows (j head-pair x qc 256-q columns):
  scores are kt-PAIRED into one [128,1024] PSUM tile per two k-tiles: the A
  (tile_position (0,0)) halves fill bank X (cols 0:512), the B ((64,0))
  halves bank Y (cols 512:1024) -- a tile_position pair sharing a bank, or
  any start at a non-bank-aligned offset, crashes the hw, but one group per
  bank (start only on the even kt, lazy zero-region covering the odd kt's
  cols) is legal. One exp (ACT) and one mask-multiply (DVE 2x bf16) then
  cover 2 heads x 2 k-tiles, halving ACT op count (64 exps total).
- AV restructured: stationary = P^T tile [128k,128q], moving = Vext [128k,65]
  (ones column) -> av [128q,65] accumulated in one bank-sized PSUM tile as a
  SINGLE accumulation group (start only on the first matmul: the lazy
  zero-region covers all 4 chains; stop only on the last). Halves AV
  tensor-engine rows (ap=65 vs 512; LdWeights is free in the cost model) and
  makes the softmax denominator a per-PARTITION column: normalization is one
  reciprocal + one broadcast multiply -- no DMA broadcasts at all.
- Each window's AV matmuls run one FULL window later (pt tiles are a window
  old, so the exp->mask chain can never stall them); att[q,d] returns to
  attT[d,q] via PE transposes (4/window) batched in a dedicated PSUM bank
  with one [128,256] eviction into attl.
- Junk warm-up matmuls ramp the PE p-state during the DMA prologue; V-chunk
  evictions alternate ACT/DVE; out-projection chunks evict to f16 (halves
  output DMA) alternating ACT/DVE, host sums partials + (bv@Wo+bo) row.
- PSUM = exactly 8 banks: scores 2x[128,1024] + av [128,512] + transpose
  batch [128,1024]bf16 + 2x[128,512] general ring (projections/V/out-proj).
"""
import os
import sys

for _p in ("/opt/trn_rl_repo", "/root/.axon_site/_ro/trn_rl_repo"):
    if os.path.isdir(_p) and _p not in sys.path:
        sys.path.insert(0, _p)

import numpy as np
import ml_dtypes

import concourse.bass as bass
from concourse import bacc
import concourse.mybir as mybir
import concourse.tile as tile
from concourse.bass_utils import run_bass_kernel_spmd
from concourse.masks import make_identity

B, S, HID, H, D = 4, 1024, 1024, 16, 64
NCORES = 8
GH = 8          # heads per core
LOC = GH * D    # 512, local hid slice
CORE_IDS = list(range(NCORES))

bf16 = mybir.dt.bfloat16
f32 = mybir.dt.float32
AF = mybir.ActivationFunctionType
ALU = mybir.AluOpType

_NC_CACHE = None


def _build(dbg=False, reps=1):
    nc = bacc.Bacc("TRN2", debug=False, num_devices=NCORES)

    xt = nc.declare_dram_parameter("xt", [128, 8, 1024], bf16, False)
    wq = nc.declare_dram_parameter("wq", [128, 4, 8, 128], bf16, False)
    wk = nc.declare_dram_parameter("wk", [128, 4, 8, 128], bf16, False)
    wv = nc.declare_dram_parameter("wv", [128, 8, 512], bf16, False)
    wo = nc.declare_dram_parameter("wo", [128, 4, 1024], bf16, False)
    mp = nc.declare_dram_parameter("mp", [4, 128, 8 * 2 * 1024], bf16, False)
    tempx = nc.declare_dram_parameter("tempx", [LOC], f32, False)
    bqv = nc.declare_dram_parameter("bqv", [LOC], f32, False)
    bkv = nc.declare_dram_parameter("bkv", [LOC], f32, False)
    f16 = mybir.dt.float16
    out = nc.declare_dram_parameter("out", [S, HID], f16, True)    # partial

    with tile.TileContext(nc) as tc:
        with (
            tc.tile_pool(name="pw", bufs=4) as pw,           # weights
            tc.tile_pool(name="pxt", bufs=1) as pxt,         # xT [128,8192] bf16
            tc.tile_pool(name="pqk", bufs=8) as pqk,         # QT/KT [128,1024] bf16
            tc.tile_pool(name="pv", bufs=8) as pv,           # Vext [128,520] bf16
            tc.tile_pool(name="ppt", bufs=10) as ppt,        # P^T kt-pair [128,1024] bf16
            tc.tile_pool(name="pmask", bufs=2) as pmask,     # mask pair [128,16384] bf16
            tc.tile_pool(name="patt", bufs=4) as patt,       # att [128,256] bf16
            tc.tile_pool(name="pattl", bufs=4) as pattl,     # attT per j (live to end)
            tc.tile_pool(name="prcp", bufs=3) as prcp,       # recip [128,4] f32
            tc.tile_pool(name="pout", bufs=4) as pout,       # out staging f16
            tc.tile_pool(name="pconst", bufs=1) as pconst,   # small tiles
            tc.tile_pool(name="psc", bufs=2, space="PSUM") as psc,   # scores [128,1024]
            tc.tile_pool(name="pav", bufs=1, space="PSUM") as pav,   # av [128,512]
            tc.tile_pool(name="ptrp", bufs=1, space="PSUM") as ptrp, # transposes
            tc.tile_pool(name="pps", bufs=2, space="PSUM") as pps,   # general ring
        ):
            for _rep in range(reps):
                # ---- small constants (gpsimd queue, ahead of masks) ----
                def load_small(name, dram):
                    t = pconst.tile([128, 4], f32, tag=name, name=name)
                    nc.gpsimd.dma_start(
                        out=t[:], in_=dram[:].rearrange("(c p) -> p c", p=128))
                    return t

                scale_t = load_small("scale", tempx)
                bq_t = load_small("bq", bqv)
                bk_t = load_small("bk", bkv)

                bqs_t = pconst.tile([128, 4], f32, tag="bqs")
                nc.vector.tensor_mul(bqs_t[:], bq_t[:], scale_t[:])
                # preload exp table during the DMA phase
                warm_t = pconst.tile([1, 4], f32, tag="warmexp")
                nc.scalar.activation(warm_t[:], scale_t[0:1, :], AF.Exp)

                # identity for PE transposes
                ident = pconst.tile([128, 128], bf16, tag="ident")
                make_identity(nc, ident[:])

                # ---- PE warm-up: junk matmuls ramp the p-state while DMAs
                # stream; their PSUM tile is write-only and recycled ----
                wrm = pconst.tile([128, 512], bf16, tag="wrm")
                nc.vector.memset(wrm[:], 0.0)
                wps = pps.tile([128, 512], f32, tag="pp", name="warmps")
                for _w in range(8):
                    nc.tensor.matmul(wps[:], wrm[:, 0:128], wrm[:],
                                     start=True, stop=True)

                # ---- bulk loads (sync queue = HWDGE) ----
                wqb = pw.tile([128, 4 * 8 * 128], bf16, tag="w", name="wq")
                wq4 = wqb[:].rearrange("p (j c n) -> p j c n", j=4, c=8)
                wkb = pw.tile([128, 4 * 8 * 128], bf16, tag="w", name="wk")
                wk4 = wkb[:].rearrange("p (j c n) -> p j c n", j=4, c=8)
                xtb = pxt.tile([128, 8 * 1024], bf16, tag="xt", name="xt")
                xt3 = xtb[:].rearrange("p (c s) -> p c s", c=8)

                nc.sync.dma_start(out=wq4[:, 0], in_=wq[:, 0])
                nc.sync.dma_start(out=xt3[:, 0:4, 0:512], in_=xt[:, 0:4, 0:512])
                nc.sync.dma_start(out=xt3[:, 4:8, 0:512], in_=xt[:, 4:8, 0:512])
                nc.sync.dma_start(out=wk4[:, 0], in_=wk[:, 0])
                nc.sync.dma_start(out=xt3[:, 0:4, 512:1024],
                                  in_=xt[:, 0:4, 512:1024])
                nc.sync.dma_start(out=xt3[:, 4:8, 512:1024],
                                  in_=xt[:, 4:8, 512:1024])
                for _j in (1, 2, 3):
                    nc.sync.dma_start(out=wq4[:, _j], in_=wq[:, _j])
                for _j in (1, 2, 3):
                    nc.sync.dma_start(out=wk4[:, _j], in_=wk[:, _j])
                wvb = pw.tile([128, 8 * 512], bf16, tag="w", name="wv")
                wv3 = wvb[:].rearrange("p (c n) -> p c n", c=8)
                nc.sync.dma_start(out=wv3[:], in_=wv[:])
                wob = pw.tile([128, 4 * 1024], bf16, tag="w", name="wo")
                wo3 = wob[:].rearrange("p (r n) -> p r n", r=4)
                nc.sync.dma_start(out=wo3[:], in_=wo[:])

                # ---- mask pair loads (gpsimd queue), kt-granular ----
                mh = [None] * 4

                def load_mask(j):
                    t = pmask.tile([128, 8 * 2 * 1024], bf16, tag="mask",
                                   name=f"mp{j}")
                    tv = t[:].rearrange("p (k x) -> p k x", k=8)
                    for kt in range(8):
                        nc.gpsimd.dma_start(
                            out=tv[:, kt],
                            in_=mp[j, :, kt * 2048:(kt + 1) * 2048])
                    mh[j] = t

                load_mask(0)
                load_mask(1)

                # ---- Q^T/K^T projection pieces ----
                qtb = [None] * 4
                ktb = [None] * 4

                def qk_alloc(j):
                    qtb[j] = pqk.tile([128, 1024], bf16, tag="qk", name=f"qt{j}")
                    ktb[j] = pqk.tile([128, 1024], bf16, tag="qk", name=f"kt{j}")

                _qk_ps = {}

                # piece 0: Q s-half0, 1: K s-half0, 2: Q s-half1, 3: K s-half1
                def qk_half(j, piece, half, pool=None, ptag="pp", pshape=None):
                    is_q = piece % 2 == 0
                    sh = piece // 2
                    wsrc = wq4 if is_q else wk4
                    key = (j, piece)
                    if half == 0:
                        _qk_ps[key] = (pool or pps).tile(
                            pshape or [128, 512], f32, tag=ptag,
                            name=f"qkps{j}_{piece}")
                    psa = _qk_ps[key][:, 0:512]
                    for c8 in range(4 * half, 4 * half + 4):
                        nc.tensor.matmul(psa, wsrc[:, j, c8, :],
                                         xt3[:, c8, sh * 512:(sh + 1) * 512],
                                         start=(c8 == 0), stop=(c8 == 7))
                    if half == 1:
                        dst = (qtb[j] if is_q else ktb[j])[:, sh * 512:(sh + 1) * 512]
                        if is_q:
                            nc.vector.tensor_scalar(
                                dst, psa, scale_t[:, j:j + 1],
                                bqs_t[:, j:j + 1], ALU.mult, ALU.add)
                        else:
                            nc.vector.tensor_scalar_add(dst, psa,
                                                        bk_t[:, j:j + 1])

                def qk_piece(j, piece, pool=None, ptag="pp", pshape=None):
                    qk_half(j, piece, 0, pool, ptag, pshape)
                    qk_half(j, piece, 1, pool, ptag, pshape)

                # ---- V projection chunk st -> Vext [128 s, 8*65] ----
                vext = [None] * 8

                def vchunk(st, pool=None, ptag="pp", pshape=None):
                    vps = (pool or pps).tile(pshape or [128, 512], f32,
                                             tag=ptag, name=f"vps{st}")
                    vps = vps[:, 0:512]
                    for c8 in range(8):
                        nc.tensor.matmul(vps, xt3[:, c8, st * 128:(st + 1) * 128],
                                         wv3[:, c8, :], start=(c8 == 0), stop=(c8 == 7))
                    vt = pv.tile([128, 520], bf16, tag="vext", name=f"vext{st}")
                    v3 = vt[:].rearrange("p (h e) -> p h e", e=65)
                    # alternate eviction engine so neither ACT nor DVE piles up
                    if st % 2 == 0:
                        nc.scalar.activation(
                            v3[:, :, 0:64],
                            vps.rearrange("p (h e) -> p h e", e=64), AF.Copy)
                    else:
                        nc.vector.tensor_copy(
                            v3[:, :, 0:64],
                            vps.rearrange("p (h e) -> p h e", e=64))
                    nc.gpsimd.memset(v3[:, :, 64:65], 1.0)
                    vext[st] = vt

                # ---- attention machinery ----
                attl = [None] * 4
                pending_tr = []   # deferred (j, qc, att_tile, widx)

                def flush_transposes():
                    if not pending_tr:
                        return
                    j, qc, att, w = pending_tr.pop(0)
                    ptr_t = ptrp.tile([128, 1024], bf16, tag="ptr",
                                     name=f"ptr{w}")
                    for h in (0, 1):
                        for qt in (0, 1):
                            nc.tensor.transpose(
                                ptr_t[h * 64:(h + 1) * 64,
                                      qt * 128:(qt + 1) * 128],
                                att[:, (h * 2 + qt) * 64:(h * 2 + qt + 1) * 64],
                                ident[:])
                    if attl[j] is None:
                        attl[j] = pattl.tile([128, 1024], bf16, tag="attl",
                                             name=f"attl{j}")
                    nc.vector.tensor_copy(attl[j][:, qc * 256:(qc + 1) * 256],
                                          ptr_t[:, 0:256])

                # pending AV state: the avs of window w run one window later,
                # by which time every pt tile is ~a full window old, so they
                # never wait on the exp/mask chain.
                pend = {}   # pts, j, qc, widx, avt

                def pend_avs(kt):
                    # one kt's worth (4 matmuls) of the previous window's AV
                    if not pend:
                        return
                    if kt == 8:
                        # normalization: per-partition reciprocal+scalar mult
                        avt, pj, pqc, pw = (pend["avt"], pend["j"],
                                            pend["qc"], pend["widx"])
                        att = patt.tile([128, 256], bf16, tag="att",
                                        name=f"att{pw}")
                        rcp = prcp.tile([128, 4], f32, tag="rcp",
                                        name=f"rcp{pw}")
                        avv = avt[:, 0:260].rearrange("p (n e) -> p n e", e=65)
                        nc.vector.reciprocal(rcp[:], avv[:, :, 64])
                        # one broadcast multiply normalizes all 4 chains
                        nc.vector.tensor_mul(
                            att[:].rearrange("p (n e) -> p n e", e=64),
                            avv[:, :, 0:64],
                            rcp[:].to_broadcast([128, 4, 64]))
                        pending_tr.append((pj, pqc, att, pw))
                        pend.clear()
                        return
                    if kt == 0:
                        # single bank-sized av tile, one accumulation group:
                        # the first matmul's start lazily zeroes the whole 2KB
                        # zero region so all 4 chains accumulate from zero.
                        pend["avt"] = pav.tile([128, 512], f32, tag="av",
                                               name=f"av{pend['widx']}")
                    avt = pend["avt"]
                    ptk = pend["pts"][kt // 2]
                    par = kt % 2
                    pj = pend["j"]
                    for h in (0, 1):
                        hh = 2 * pj + h
                        for qt in (0, 1):
                            c = (h * 2 + qt) * 65
                            nc.tensor.matmul(
                                avt[:, c:c + 65],
                                ptk[:, h * 512 + par * 256 + qt * 128:
                                    h * 512 + par * 256 + (qt + 1) * 128],
                                vext[kt][:, hh * 65:(hh + 1) * 65],
                                start=(kt == 0 and h == 0 and qt == 0),
                                stop=(kt == 7 and h == 1 and qt == 1),
                                skip_group_check=True)

                def window(j, qc, widx, fillers):
                    qs = slice(qc * 256, (qc + 1) * 256)
                    # mask view permuted to (h, kt, q) to match pt layout
                    mpv = mh[j][:].rearrange("p (k h q) -> p h k q", k=8, h=2)
                    pts = [None] * 4
                    ps = None
                    for kt in range(9):
                        if kt < 8:
                            # kt-PAIRED scores: one [128,1024] tile per kt
                            # pair; A halves (tile_position (0,0)) in bank X
                            # (cols 0:512), B halves ((64,0)) in bank Y (cols
                            # 512:1024). Only the even kt starts the group --
                            # its lazy zero region covers the odd kt's cols.
                            par = kt % 2
                            if par == 0:
                                ps = psc.tile([128, 1024], f32, tag="sc",
                                              name=f"sc{widx}_{kt // 2}")
                            nc.tensor.matmul(
                                ps[:, par * 256:(par + 1) * 256],
                                ktb[j][0:64, kt * 128:(kt + 1) * 128],
                                qtb[j][0:64, qs],
                                start=(par == 0), stop=(par == 1),
                                tile_position=(0, 0), skip_group_check=True)
                            nc.tensor.matmul(
                                ps[:, 512 + par * 256:512 + (par + 1) * 256],
                                ktb[j][64:128, kt * 128:(kt + 1) * 128],
                                qtb[j][64:128, qs],
                                start=(par == 0), stop=(par == 1),
                                tile_position=(64, 0), skip_group_check=True)
                        pend_avs(kt)
                        for u in fillers[kt]:
                            u()
                        if kt < 8 and kt % 2 == 1:
                            # one exp + one mask multiply per kt PAIR
                            pt = ppt.tile([128, 1024], bf16, tag="pt",
                                          name=f"pt{widx}_{kt // 2}")
                            nc.scalar.activation(pt[:], ps[:], AF.Exp)
                            nc.vector.tensor_mul(
                                pt[:].rearrange("p (h k q) -> p h k q",
                                                h=2, k=2),
                                pt[:].rearrange("p (h k q) -> p h k q",
                                                h=2, k=2),
                                mpv[:, :, kt - 1:kt + 1, qs])
                            pts[kt // 2] = pt
                    pend.update(pts=pts, j=j, qc=qc, widx=widx)

                # ---- out projection chunk (qt, ch): PSUM -> f16 -> DRAM ----
                def outproj(qt, ch):
                    ops = pps.tile([128, 512], f32, tag="pp",
                                   name=f"ops{qt}_{ch}")
                    for rcx in range(4):
                        nc.tensor.matmul(ops[:],
                                         attl[rcx][:, qt * 128:(qt + 1) * 128],
                                         wo3[:, rcx, ch * 512:(ch + 1) * 512],
                                         start=(rcx == 0), stop=(rcx == 3))
                    ot = pout.tile([128, 512], f16, tag="out",
                                   name=f"ot{qt}_{ch}")
                    if ch == 0:
                        nc.vector.tensor_copy(ot[:], ops[:])
                    else:
                        nc.scalar.activation(ot[:], ops[:], AF.Copy)
                    nc.sync.dma_start(
                        out=out[qt * 128:(qt + 1) * 128, ch * 512:(ch + 1) * 512],
                        in_=ot[:])

                # ---- schedule ----
                # head chains spread over the (still idle) psc/pav banks so
                # the 2-deep pps ring doesn't serialize the prologue
                qk_alloc(0)
                qk_piece(0, 0)
                qk_piece(0, 1, psc, "sc", [128, 1024])
                qk_piece(0, 2)
                qk_piece(0, 3, pav, "av")
                vchunk(0)
                vchunk(1)

                _vps = {}

                def vA(st):
                    def g():
                        _vps[st] = pps.tile([128, 512], f32, tag="pp",
                                            name=f"vps{st}")
                        for c8 in range(4):
                            nc.tensor.matmul(
                                _vps[st][:], xt3[:, c8, st * 128:(st + 1) * 128],
                                wv3[:, c8, :], start=(c8 == 0), stop=False)
                    return g

                def vB(st):
                    def g():
                        vps = _vps[st]
                        for c8 in range(4, 8):
                            nc.tensor.matmul(
                                vps[:], xt3[:, c8, st * 128:(st + 1) * 128],
                                wv3[:, c8, :], start=False, stop=(c8 == 7))
                        vt = pv.tile([128, 520], bf16, tag="vext",
                                     name=f"vext{st}")
                        v3 = vt[:].rearrange("p (h e) -> p h e", e=65)
                        if st % 2 == 0:
                            nc.scalar.activation(
                                v3[:, :, 0:64],
                                vps[:].rearrange("p (h e) -> p h e", e=64),
                                AF.Copy)
                        else:
                            nc.vector.tensor_copy(
                                v3[:, :, 0:64],
                                vps[:].rearrange("p (h e) -> p h e", e=64))
                        nc.gpsimd.memset(v3[:, :, 64:65], 1.0)
                        vext[st] = vt
                    return g

                def qkh(j, piece, half):
                    def g():
                        if piece == 0 and half == 0:
                            qk_alloc(j)
                        qk_half(j, piece, half)
                    return g

                def op(qt, ch):
                    return lambda: outproj(qt, ch)

                T = flush_transposes

                def slots(*units_at):
                    # units_at: dict slot -> list of units
                    f = [[] for _ in range(9)]
                    for s, us in units_at:
                        f[s].extend(us)
                    return f

                # per-window filler plans. Constraints: vext[kt] ready before
                # window 1 slot kt (all vchunks inside window 0); Q/K sh0 of
                # pair j evicted before window (j,0), K sh1 before its slot 4,
                # Q sh1 before (j,2); out-proj chunks once their attl quarter
                # (evicted 2 windows after the producing window) exists.
                F = [
                    slots((0, [vA(2)]), (1, [vB(2)]), (2, [vA(3)]),
                          (3, [vB(3)]), (4, [vA(4)]), (5, [vB(4)]),
                          (6, [vA(5)]), (7, [vB(5)])),
                    slots((0, [T]), (1, [vA(6)]), (2, [vB(6)]),
                          (3, [vA(7)]), (4, [vB(7)]),
                          (5, [qkh(1, 0, 0)]), (7, [qkh(1, 0, 1)])),
                    slots((0, [T]), (1, [qkh(1, 1, 0)]), (4, [qkh(1, 1, 1)])),
                    slots((0, [T]), (1, [qkh(1, 3, 0)]), (4, [qkh(1, 3, 1)])),
                    slots((0, [T]), (1, [qkh(1, 2, 0)]), (4, [qkh(1, 2, 1)])),
                    slots((0, [T]), (1, [qkh(2, 0, 0)]), (4, [qkh(2, 0, 1)])),
                    slots((0, [T]), (1, [qkh(2, 1, 0)]), (4, [qkh(2, 1, 1)])),
                    slots((0, [T]), (1, [qkh(2, 3, 0)]), (4, [qkh(2, 3, 1)])),
                    slots((0, [T]), (1, [qkh(2, 2, 0)]), (4, [qkh(2, 2, 1)])),
                    slots((0, [T]), (1, [qkh(3, 0, 0)]), (4, [qkh(3, 0, 1)])),
                    slots((0, [T]), (1, [qkh(3, 1, 0)]), (4, [qkh(3, 1, 1)])),
                    slots((0, [T]), (1, [qkh(3, 3, 0)]), (4, [qkh(3, 3, 1)])),
                    slots((0, [T]), (1, [qkh(3, 2, 0)]), (4, [qkh(3, 2, 1)])),
                    slots((0, [T])),
                    slots((0, [T]), (1, [op(0, 0)]), (3, [op(0, 1)]),
                          (5, [op(1, 0)]), (7, [op(1, 1)])),
                    slots((0, [T]), (1, [op(2, 0)]), (3, [op(2, 1)]),
                          (5, [op(3, 0)]), (7, [op(3, 1)])),
                ]

                widx = 0
                for j in range(4):
                    for qc in range(4):
                        window(j, qc, widx, F[widx])
                        widx += 1
                        if (j, qc) == (0, 1):
                            load_mask(2)
                        elif (j, qc) == (1, 1):
                            load_mask(3)

                # tail: flush window 14's transposes (enables qt 4/5 chunks),
                # run the final window's AVs interleaved with those chunks,
                # then its norm + transposes and the last out-proj chunks.
                flush_transposes()   # att of window 14 -> attl[3] cols 512:768
                tail_ops = [op(4, 0), op(4, 1), op(5, 0), op(5, 1)]
                for kt in range(9):
                    pend_avs(kt)
                    if kt in (1, 3, 5, 7):
                        tail_ops.pop(0)()
                flush_transposes()   # att of window 15 -> attl[3] cols 768:
                outproj(6, 0)
                outproj(6, 1)
                outproj(7, 0)
                outproj(7, 1)

    nc.compile()
    return nc


def _get_nc():
    global _NC_CACHE
    if _NC_CACHE is None:
        _NC_CACHE = _build()
    return _NC_CACHE


def _prep_inputs(x, Wq, bq, Wk, bk, Wv, bv, Wo, bo, temperature, sparse_mask):
    bfd = ml_dtypes.bfloat16
    x = np.asarray(x, np.float32)
    Wq = np.asarray(Wq, np.float32); Wk = np.asarray(Wk, np.float32)
    Wv = np.asarray(Wv, np.float32); Wo = np.asarray(Wo, np.float32)
    bq = np.asarray(bq, np.float32); bk = np.asarray(bk, np.float32)
    temp = np.asarray(temperature, np.float32).reshape(-1)
    mask = np.asarray(sparse_mask)

    in_maps = []
    for c in CORE_IDS:
        b, g = c // 2, c % 2
        cols = slice(g * LOC, (g + 1) * LOC)
        hs = slice(g * GH, (g + 1) * GH)
        xt_h = np.ascontiguousarray(
            x[b].T.reshape(8, 128, 1024).transpose(1, 0, 2)).astype(bfd)
        wq_h = np.ascontiguousarray(
            Wq[:, cols].reshape(8, 128, 4, 128).transpose(1, 2, 0, 3)).astype(bfd)
        wk_h = np.ascontiguousarray(
            Wk[:, cols].reshape(8, 128, 4, 128).transpose(1, 2, 0, 3)).astype(bfd)
        wv_h = np.ascontiguousarray(
            Wv[:, cols].reshape(8, 128, 512).transpose(1, 0, 2)).astype(bfd)
        wo_h = np.ascontiguousarray(
            Wo[cols, :].reshape(4, 128, 1024).transpose(1, 0, 2)).astype(bfd)
        # mask pairs: [4 j, 128 p, 8 kt, 2 h, 1024 q]; element (j,p,kt,h,q) =
        # sparse_mask[b, hs[2j+h], q, kt*128+p]
        mt = mask[b, hs].transpose(0, 2, 1)            # [8h, 1024k, 1024q]
        mp_h = np.ascontiguousarray(
            mt.reshape(4, 2, 8, 128, 1024).transpose(0, 3, 2, 1, 4)
        ).astype(bfd).reshape(4, 128, 16384)
        in_maps.append({
            "xt": xt_h, "wq": wq_h, "wk": wk_h, "wv": wv_h, "wo": wo_h,
            "mp": mp_h,
            "tempx": (np.repeat(temp[hs], D) / np.sqrt(D)).astype(np.float32),
            "bqv": np.ascontiguousarray(bq[cols]),
            "bkv": np.ascontiguousarray(bk[cols]),
        })
    return in_maps


def kernel(**inputs):
    in_maps = _prep_inputs(**inputs)
    nc = _get_nc()
    res = run_bass_kernel_spmd(nc, in_maps, CORE_IDS)
    # unshard: row-parallel partial sum per batch + constant bias row
    # (softmax rows sum to 1 so bv contributes bv @ Wo to every row)
    bv = np.asarray(inputs["bv"], np.float32)
    bo = np.asarray(inputs["bo"], np.float32)
    Wo = np.asarray(inputs["Wo"], np.float32)
    brow = bv @ Wo + bo
    out = np.empty((B, S, HID), np.float32)
    for b in range(B):
        out[b] = (res.results[2 * b]["out"].astype(np.float32)
                  + res.results[2 * b + 1]["out"].astype(np.float32) + brow)
    return out


# revision 58
# speedup vs baseline: 1.0747x; 1.0747x over previous
"""Trainium2 Bass kernel for nn_AdaptiveAttention (sparse attention, B=4 S=1024 HID=1024 H=16).

Sharding (8 cores): core c = (batch b=c//2) x (head-group g=c%2, 8 heads / 512 hid cols).

v2 design (cost-model driven; ~130.7us vs 151.9us v1 baseline):
- All DRAM inputs host-pre-tiled into exact SBUF layouts so every DMA is a
  contiguous >=1KB-run burst (full-rate in the DMA model; elem runs >=512B).
- Q^T/K^T = W x x^T with temperature/sqrt(D) folded into the Q eviction
  (DVE tensor_scalar).
- Attention runs in 16 quarter-windows (j head-pair x qc 256-q columns):
  scores are kt-PAIRED into one [128,1024] PSUM tile per two k-tiles: the A
  (tile_position (0,0)) halves fill bank X (cols 0:512), the B ((64,0))
  halves bank Y (cols 512:1024) -- a tile_position pair sharing a bank, or
  any start at a non-bank-aligned offset, crashes the hw, but one group per
  bank (start only on the even kt, lazy zero-region covering the odd kt's
  cols) is legal. One exp (ACT) and one mask-multiply (DVE 2x bf16) then
  cover 2 heads x 2 k-tiles, halving ACT op count (64 exps total).
- AV restructured: stationary = P^T tile [128k,128q], moving = Vext [128k,65]
  (ones column) -> av [128q,65] accumulated in one bank-sized PSUM tile as a
  SINGLE accumulation group (start only on the first matmul: the lazy
  zero-region covers all 4 chains; stop only on the last). Halves AV
  tensor-engine rows (ap=65 vs 512; LdWeights is free in the cost model) and
  makes the softmax denominator a per-PARTITION column: normalization is one
  reciprocal + one broadcast multiply -- no DMA broadcasts at all.
- Each window's AV matmuls run one FULL window later (pt tiles are a window
  old, so the exp->mask chain can never stall them); att[q,d] returns to
  attT[d,q] via PE transposes (4/window) batched in a dedicated PSUM bank
  with one [128,256] eviction into attl.
- Junk warm-up matmuls ramp the PE p-state during the DMA prologue; V-chunk
  evictions alternate ACT/DVE; out-projection chunks evict to f16 (halves
  output DMA) alternating ACT/DVE, host sums partials + (bv@Wo+bo) row.
- PSUM = exactly 8 banks: scores 2x[128,1024] + av [128,512] + transpose
  batch [128,1024]bf16 + 2x[128,512] general ring (projections/V/out-proj).
"""
import os
import sys

for _p in ("/opt/trn_rl_repo", "/root/.axon_site/_ro/trn_rl_repo"):
    if os.path.isdir(_p) and _p not in sys.path:
        sys.path.insert(0, _p)

import numpy as np
import ml_dtypes

import concourse.bass as bass
from concourse import bacc
import concourse.mybir as mybir
import concourse.tile as tile
from concourse.bass_utils import run_bass_kernel_spmd
from concourse.masks import make_identity

B, S, HID, H, D = 4, 1024, 1024, 16, 64
NCORES = 8
GH = 8          # heads per core
LOC = GH * D    # 512, local hid slice
CORE_IDS = list(range(NCORES))

bf16 = mybir.dt.bfloat16
f32 = mybir.dt.float32
AF = mybir.ActivationFunctionType
ALU = mybir.AluOpType

_NC_CACHE = None


def _build(dbg=False, reps=1):
    nc = bacc.Bacc("TRN2", debug=False, num_devices=NCORES)

    xt = nc.declare_dram_parameter("xt", [128, 8, 1024], bf16, False)
    wq = nc.declare_dram_parameter("wq", [128, 4, 8, 128], bf16, False)
    wk = nc.declare_dram_parameter("wk", [128, 4, 8, 128], bf16, False)
    wv = nc.declare_dram_parameter("wv", [128, 8, 512], bf16, False)
    wo = nc.declare_dram_parameter("wo", [128, 4, 1024], bf16, False)
    mp = nc.declare_dram_parameter("mp", [4, 128, 8 * 2 * 1024], bf16, False)
    tempx = nc.declare_dram_parameter("tempx", [LOC], f32, False)
    bqv = nc.declare_dram_parameter("bqv", [LOC], f32, False)
    bkv = nc.declare_dram_parameter("bkv", [LOC], f32, False)
    f16 = mybir.dt.float16
    out = nc.declare_dram_parameter("out", [S, HID], f16, True)    # partial

    with tile.TileContext(nc) as tc:
        with (
            tc.tile_pool(name="pw", bufs=4) as pw,           # weights
            tc.tile_pool(name="pxt", bufs=1) as pxt,         # xT [128,8192] bf16
            tc.tile_pool(name="pqk", bufs=8) as pqk,         # QT/KT [128,1024] bf16
            tc.tile_pool(name="pv", bufs=8) as pv,           # Vext [128,520] bf16
            tc.tile_pool(name="ppt", bufs=10) as ppt,        # P^T kt-pair [128,1024] bf16
            tc.tile_pool(name="pmask", bufs=2) as pmask,     # mask pair [128,16384] bf16
            tc.tile_pool(name="patt", bufs=4) as patt,       # att [128,256] bf16
            tc.tile_pool(name="pattl", bufs=4) as pattl,     # attT per j (live to end)
            tc.tile_pool(name="prcp", bufs=3) as prcp,       # recip [128,4] f32
            tc.tile_pool(name="pout", bufs=4) as pout,       # out staging f16
            tc.tile_pool(name="pconst", bufs=1) as pconst,   # small tiles
            tc.tile_pool(name="psc", bufs=2, space="PSUM") as psc,   # scores [128,1024]
            tc.tile_pool(name="pav", bufs=1, space="PSUM") as pav,   # av [128,512]
            tc.tile_pool(name="ptrp", bufs=1, space="PSUM") as ptrp, # transposes
            tc.tile_pool(name="pps", bufs=2, space="PSUM") as pps,   # general ring
        ):
            for _rep in range(reps):
                # ---- small constants (gpsimd queue, ahead of masks) ----
                def load_small(name, dram):
                    t = pconst.tile([128, 4], f32, tag=name, name=name)
                    nc.sync.dma_start(
                        out=t[:], in_=dram[:].rearrange("(c p) -> p c", p=128))
                    return t

                # warm-up memset first: no deps, so the PE warm-up isn't
                # queued behind the const DMAs on DVE
                wrm = pconst.tile([128, 512], bf16, tag="wrm")
                nc.vector.memset(wrm[:], 0.0)

                # identity for PE transposes
                ident = pconst.tile([128, 128], bf16, tag="ident")
                make_identity(nc, ident[:])

                # ---- PE warm-up: junk matmuls ramp the p-state while DMAs
                # stream; their PSUM tile is write-only and recycled ----
                wps = pps.tile([128, 512], f32, tag="pp", name="warmps")
                for _w in range(20):
                    nc.tensor.matmul(wps[:], wrm[:, 0:128], wrm[:],
                                     start=True, stop=True)

                # ---- bulk loads (sync queue = HWDGE) ----
                wqb = pw.tile([128, 4 * 8 * 128], bf16, tag="w", name="wq")
                wq4 = wqb[:].rearrange("p (j c n) -> p j c n", j=4, c=8)
                wkb = pw.tile([128, 4 * 8 * 128], bf16, tag="w", name="wk")
                wk4 = wkb[:].rearrange("p (j c n) -> p j c n", j=4, c=8)
                xtb = pxt.tile([128, 8 * 1024], bf16, tag="xt", name="xt")
                xt3 = xtb[:].rearrange("p (c s) -> p c s", c=8)

                scale_t = load_small("scale", tempx)
                bq_t = load_small("bq", bqv)
                bk_t = load_small("bk", bkv)
                bqs_t = pconst.tile([128, 4], f32, tag="bqs")
                nc.vector.tensor_mul(bqs_t[:], bq_t[:], scale_t[:])
                # preload exp table during the DMA phase
                warm_t = pconst.tile([1, 4], f32, tag="warmexp")
                nc.scalar.activation(warm_t[:], scale_t[0:1, :], AF.Exp)
                nc.sync.dma_start(out=wq4[:, 0], in_=wq[:, 0])
                nc.sync.dma_start(out=xt3[:, 0:4, 0:512], in_=xt[:, 0:4, 0:512])
                nc.sync.dma_start(out=xt3[:, 4:8, 0:512], in_=xt[:, 4:8, 0:512])
                nc.sync.dma_start(out=wk4[:, 0], in_=wk[:, 0])
                nc.sync.dma_start(out=xt3[:, 0:4, 512:1024],
                                  in_=xt[:, 0:4, 512:1024])
                nc.sync.dma_start(out=xt3[:, 4:8, 512:1024],
                                  in_=xt[:, 4:8, 512:1024])
                wvb = pw.tile([128, 8 * 512], bf16, tag="w", name="wv")
                wv3 = wvb[:].rearrange("p (c n) -> p c n", c=8)
                nc.sync.dma_start(out=wv3[:], in_=wv[:])
                for _j in (1, 2, 3):
                    nc.sync.dma_start(out=wq4[:, _j], in_=wq[:, _j])
                for _j in (1, 2, 3):
                    nc.sync.dma_start(out=wk4[:, _j], in_=wk[:, _j])
                wob = pw.tile([128, 4 * 1024], bf16, tag="w", name="wo")
                wo3 = wob[:].rearrange("p (r n) -> p r n", r=4)
                nc.sync.dma_start(out=wo3[:], in_=wo[:])

                # ---- mask pair loads (gpsimd queue), kt-granular ----
                mh = [None] * 4

                def load_mask(j):
                    t = pmask.tile([128, 8 * 2 * 1024], bf16, tag="mask",
                                   name=f"mp{j}")
                    tv = t[:].rearrange("p (k x) -> p k x", k=8)
                    for kt in range(8):
                        nc.gpsimd.dma_start(
                            out=tv[:, kt],
                            in_=mp[j, :, kt * 2048:(kt + 1) * 2048])
                    mh[j] = t

                load_mask(0)
                load_mask(1)

                # ---- Q^T/K^T projection pieces ----
                qtb = [None] * 4
                ktb = [None] * 4

                def qk_alloc(j):
                    qtb[j] = pqk.tile([128, 1024], bf16, tag="qk", name=f"qt{j}")
                    ktb[j] = pqk.tile([128, 1024], bf16, tag="qk", name=f"kt{j}")

                _qk_ps = {}

                # piece 0: Q s-half0, 1: K s-half0, 2: Q s-half1, 3: K s-half1
                def qk_half(j, piece, half, pool=None, ptag="pp", pshape=None):
                    is_q = piece % 2 == 0
                    sh = piece // 2
                    wsrc = wq4 if is_q else wk4
                    key = (j, piece)
                    if half == 0:
                        _qk_ps[key] = (pool or pps).tile(
                            pshape or [128, 512], f32, tag=ptag,
                            name=f"qkps{j}_{piece}")
                    psa = _qk_ps[key][:, 0:512]
                    for c8 in range(4 * half, 4 * half + 4):
                        nc.tensor.matmul(psa, wsrc[:, j, c8, :],
                                         xt3[:, c8, sh * 512:(sh + 1) * 512],
                                         start=(c8 == 0), stop=(c8 == 7))
                    if half == 1:
                        dst = (qtb[j] if is_q else ktb[j])[:, sh * 512:(sh + 1) * 512]
                        if is_q:
                            nc.vector.tensor_scalar(
                                dst, psa, scale_t[:, j:j + 1],
                                bqs_t[:, j:j + 1], ALU.mult, ALU.add)
                        else:
                            nc.vector.tensor_scalar_add(dst, psa,
                                                        bk_t[:, j:j + 1])

                def qk_piece(j, piece, pool=None, ptag="pp", pshape=None):
                    qk_half(j, piece, 0, pool, ptag, pshape)
                    qk_half(j, piece, 1, pool, ptag, pshape)

                # ---- V projection chunk st -> Vext [128 s, 8*65] ----
                vext = [None] * 8

                def vchunk(st, pool=None, ptag="pp", pshape=None):
                    vps = (pool or pps).tile(pshape or [128, 512], f32,
                                             tag=ptag, name=f"vps{st}")
                    vps = vps[:, 0:512]
                    for c8 in range(8):
                        nc.tensor.matmul(vps, xt3[:, c8, st * 128:(st + 1) * 128],
                                         wv3[:, c8, :], start=(c8 == 0), stop=(c8 == 7))
                    vt = pv.tile([128, 520], bf16, tag="vext", name=f"vext{st}")
                    v3 = vt[:].rearrange("p (h e) -> p h e", e=65)
                    # alternate eviction engine so neither ACT nor DVE piles up
                    if st % 2 == 0:
                        nc.scalar.activation(
                            v3[:, :, 0:64],
                            vps.rearrange("p (h e) -> p h e", e=64), AF.Copy)
                    else:
                        nc.vector.tensor_copy(
                            v3[:, :, 0:64],
                            vps.rearrange("p (h e) -> p h e", e=64))
                    nc.gpsimd.memset(v3[:, :, 64:65], 1.0)
                    vext[st] = vt

                # ---- attention machinery ----
                attl = [None] * 4
                pending_tr = []   # deferred (j, qc, att_tile, widx)

                def flush_transposes():
                    if not pending_tr:
                        return
                    j, qc, att, w = pending_tr.pop(0)
                    ptr_t = ptrp.tile([128, 1024], bf16, tag="ptr",
                                     name=f"ptr{w}")
                    for h in (0, 1):
                        for qt in (0, 1):
                            nc.tensor.transpose(
                                ptr_t[h * 64:(h + 1) * 64,
                                      qt * 128:(qt + 1) * 128],
                                att[:, (h * 2 + qt) * 64:(h * 2 + qt + 1) * 64],
                                ident[:])
                    if attl[j] is None:
                        attl[j] = pattl.tile([128, 1024], bf16, tag="attl",
                                             name=f"attl{j}")
                    nc.vector.tensor_copy(attl[j][:, qc * 256:(qc + 1) * 256],
                                          ptr_t[:, 0:256])

                # pending AV state: the avs of window w run one window later,
                # by which time every pt tile is ~a full window old, so they
                # never wait on the exp/mask chain.
                pend = {}   # pts, j, qc, widx, avt

                def pend_avs(kt):
                    # one kt's worth (4 matmuls) of the previous window's AV
                    if not pend:
                        return
                    if kt == 8:
                        # normalization: per-partition reciprocal+scalar mult
                        avt, pj, pqc, pw = (pend["avt"], pend["j"],
                                            pend["qc"], pend["widx"])
                        att = patt.tile([128, 256], bf16, tag="att",
                                        name=f"att{pw}")
                        rcp = prcp.tile([128, 4], f32, tag="rcp",
                                        name=f"rcp{pw}")
                        avv = avt[:, 0:260].rearrange("p (n e) -> p n e", e=65)
                        nc.vector.reciprocal(rcp[:], avv[:, :, 64])
                        # one broadcast multiply normalizes all 4 chains
                        nc.vector.tensor_mul(
                            att[:].rearrange("p (n e) -> p n e", e=64),
                            avv[:, :, 0:64],
                            rcp[:].to_broadcast([128, 4, 64]))
                        pending_tr.append((pj, pqc, att, pw))
                        pend.clear()
                        return
                    if kt == 0:
                        # single bank-sized av tile, one accumulation group:
                        # the first matmul's start lazily zeroes the whole 2KB
                        # zero region so all 4 chains accumulate from zero.
                        pend["avt"] = pav.tile([128, 512], f32, tag="av",
                                               name=f"av{pend['widx']}")
                    avt = pend["avt"]
                    ptk = pend["pts"][kt // 2]
                    par = kt % 2
                    pj = pend["j"]
                    for h in (0, 1):
                        hh = 2 * pj + h
                        for qt in (0, 1):
                            c = (h * 2 + qt) * 65
                            nc.tensor.matmul(
                                avt[:, c:c + 65],
                                ptk[:, h * 512 + par * 256 + qt * 128:
                                    h * 512 + par * 256 + (qt + 1) * 128],
                                vext[kt][:, hh * 65:(hh + 1) * 65],
                                start=(kt == 0 and h == 0 and qt == 0),
                                stop=(kt == 7 and h == 1 and qt == 1),
                                skip_group_check=True)

                def window(j, qc, widx, fillers):
                    qs = slice(qc * 256, (qc + 1) * 256)
                    # mask view permuted to (h, kt, q) to match pt layout
                    mpv = mh[j][:].rearrange("p (k h q) -> p h k q", k=8, h=2)
                    pts = [None] * 4
                    ps = None
                    for kt in range(9):
                        if kt < 8:
                            # kt-PAIRED scores: one [128,1024] tile per kt
                            # pair; A halves (tile_position (0,0)) in bank X
                            # (cols 0:512), B halves ((64,0)) in bank Y (cols
                            # 512:1024). Only the even kt starts the group --
                            # its lazy zero region covers the odd kt's cols.
                            par = kt % 2
                            if par == 0:
                                ps = psc.tile([128, 1024], f32, tag="sc",
                                              name=f"sc{widx}_{kt // 2}")
                            nc.tensor.matmul(
                                ps[:, par * 256:(par + 1) * 256],
                                ktb[j][0:64, kt * 128:(kt + 1) * 128],
                                qtb[j][0:64, qs],
                                start=(par == 0), stop=(par == 1),
                                tile_position=(0, 0), skip_group_check=True)
                            nc.tensor.matmul(
                                ps[:, 512 + par * 256:512 + (par + 1) * 256],
                                ktb[j][64:128, kt * 128:(kt + 1) * 128],
                                qtb[j][64:128, qs],
                                start=(par == 0), stop=(par == 1),
                                tile_position=(64, 0), skip_group_check=True)
                        pend_avs(kt)
                        for u in fillers[kt]:
                            u()
                        if kt < 8 and kt % 2 == 1:
                            # one exp + one mask multiply per kt PAIR
                            pt = ppt.tile([128, 1024], bf16, tag="pt",
                                          name=f"pt{widx}_{kt // 2}")
                            nc.scalar.activation(pt[:], ps[:], AF.Exp)
                            nc.vector.tensor_mul(
                                pt[:].rearrange("p (h k q) -> p h k q",
                                                h=2, k=2),
                                pt[:].rearrange("p (h k q) -> p h k q",
                                                h=2, k=2),
                                mpv[:, :, kt - 1:kt + 1, qs])
                            pts[kt // 2] = pt
                    pend.update(pts=pts, j=j, qc=qc, widx=widx)

                # ---- out projection chunk (qt, ch): PSUM -> f16 -> DRAM ----
                def outproj(qt, ch):
                    ops = pps.tile([128, 512], f32, tag="pp",
                                   name=f"ops{qt}_{ch}")
                    for rcx in range(4):
                        nc.tensor.matmul(ops[:],
                                         attl[rcx][:, qt * 128:(qt + 1) * 128],
                                         wo3[:, rcx, ch * 512:(ch + 1) * 512],
                                         start=(rcx == 0), stop=(rcx == 3))
                    ot = pout.tile([128, 512], f16, tag="out",
                                   name=f"ot{qt}_{ch}")
                    if ch == 0:
                        nc.vector.tensor_copy(ot[:], ops[:])
                    else:
                        nc.scalar.activation(ot[:], ops[:], AF.Copy)
                    nc.sync.dma_start(
                        out=out[qt * 128:(qt + 1) * 128, ch * 512:(ch + 1) * 512],
                        in_=ot[:])

                # ---- schedule ----
                # head chains spread over the (still idle) psc/pav banks so
                # the 2-deep pps ring doesn't serialize the prologue
                qk_alloc(0)
                qk_piece(0, 0)
                qk_piece(0, 1, psc, "sc", [128, 1024])
                qk_piece(0, 2)
                qk_piece(0, 3, pav, "av")
                vchunk(0)
                vchunk(1)

                _vps = {}

                def vA(st):
                    def g():
                        _vps[st] = pps.tile([128, 512], f32, tag="pp",
                                            name=f"vps{st}")
                        for c8 in range(4):
                            nc.tensor.matmul(
                                _vps[st][:], xt3[:, c8, st * 128:(st + 1) * 128],
                                wv3[:, c8, :], start=(c8 == 0), stop=False)
                    return g

                def vB(st):
                    def g():
                        vps = _vps[st]
                        for c8 in range(4, 8):
                            nc.tensor.matmul(
                                vps[:], xt3[:, c8, st * 128:(st + 1) * 128],
                                wv3[:, c8, :], start=False, stop=(c8 == 7))
                        vt = pv.tile([128, 520], bf16, tag="vext",
                                     name=f"vext{st}")
                        v3 = vt[:].rearrange("p (h e) -> p h e", e=65)
                        if st % 2 == 0:
                            nc.scalar.activation(
                                v3[:, :, 0:64],
                                vps[:].rearrange("p (h e) -> p h e", e=64),
                                AF.Copy)
                        else:
                            nc.vector.tensor_copy(
                                v3[:, :, 0:64],
                                vps[:].rearrange("p (h e) -> p h e", e=64))
                        nc.gpsimd.memset(v3[:, :, 64:65], 1.0)
                        vext[st] = vt
                    return g

                def qkh(j, piece, half):
                    def g():
                        if piece == 0 and half == 0:
                            qk_alloc(j)
                        qk_half(j, piece, half)
                    return g

                def op(qt, ch):
                    return lambda: outproj(qt, ch)

                T = flush_transposes

                def slots(*units_at):
                    # units_at: dict slot -> list of units
                    f = [[] for _ in range(9)]
                    for s, us in units_at:
                        f[s].extend(us)
                    return f

                # per-window filler plans. Constraints: vext[kt] ready before
                # window 1 slot kt (all vchunks inside window 0); Q/K sh0 of
                # pair j evicted before window (j,0), K sh1 before its slot 4,
                # Q sh1 before (j,2); out-proj chunks once their attl quarter
                # (evicted 2 windows after the producing window) exists.
                F = [
                    slots((0, [vA(2)]), (1, [vB(2)]), (2, [vA(3)]),
                          (3, [vB(3)]), (4, [vA(4)]), (5, [vB(4)]),
                          (6, [vA(5)]), (7, [vB(5)])),
                    slots((0, [T]), (1, [vA(6)]), (2, [vB(6)]),
                          (3, [vA(7)]), (4, [vB(7)]),
                          (5, [qkh(1, 0, 0)]), (7, [qkh(1, 0, 1)])),
                    slots((0, [T]), (1, [qkh(1, 1, 0)]), (4, [qkh(1, 1, 1)])),
                    slots((0, [T]), (1, [qkh(1, 3, 0)]), (4, [qkh(1, 3, 1)])),
                    slots((0, [T]), (1, [qkh(1, 2, 0)]), (4, [qkh(1, 2, 1)])),
                    slots((0, [T]), (1, [qkh(2, 0, 0)]), (4, [qkh(2, 0, 1)])),
                    slots((0, [T]), (1, [qkh(2, 1, 0)]), (4, [qkh(2, 1, 1)])),
                    slots((0, [T]), (1, [qkh(2, 3, 0)]), (4, [qkh(2, 3, 1)])),
                    slots((0, [T]), (1, [qkh(2, 2, 0)]), (4, [qkh(2, 2, 1)])),
                    slots((0, [T]), (1, [qkh(3, 0, 0)]), (4, [qkh(3, 0, 1)])),
                    slots((0, [T]), (1, [qkh(3, 1, 0)]), (4, [qkh(3, 1, 1)])),
                    slots((0, [T]), (1, [qkh(3, 3, 0)]), (4, [qkh(3, 3, 1)])),
                    slots((0, [T]), (1, [qkh(3, 2, 0)]), (4, [qkh(3, 2, 1)])),
                    slots((0, [T])),
                    slots((0, [T]), (1, [op(0, 0)]), (3, [op(0, 1)]),
                          (5, [op(1, 0)]), (7, [op(1, 1)])),
                    slots((0, [T]), (1, [op(2, 0)]), (3, [op(2, 1)]),
                          (5, [op(3, 0)]), (7, [op(3, 1)])),
                ]

                widx = 0
                for j in range(4):
                    for qc in range(4):
                        window(j, qc, widx, F[widx])
                        widx += 1
                        if (j, qc) == (0, 1):
                            load_mask(2)
                        elif (j, qc) == (1, 1):
                            load_mask(3)

                # tail: flush window 14's transposes (enables qt 4/5 chunks),
                # run the final window's AVs interleaved with those chunks,
                # then its norm + transposes and the last out-proj chunks.
                flush_transposes()   # att of window 14 -> attl[3] cols 512:768
                tail_ops = [op(4, 0), op(4, 1), op(5, 0), op(5, 1)]
                for kt in range(9):
                    pend_avs(kt)
                    if kt in (1, 3, 5, 7):
                        tail_ops.pop(0)()
                flush_transposes()   # att of window 15 -> attl[3] cols 768:
                outproj(6, 0)
                outproj(6, 1)
                outproj(7, 0)
                outproj(7, 1)

    nc.compile()
    return nc


def _get_nc():
    global _NC_CACHE
    if _NC_CACHE is None:
        _NC_CACHE = _build()
    return _NC_CACHE


def _prep_inputs(x, Wq, bq, Wk, bk, Wv, bv, Wo, bo, temperature, sparse_mask):
    bfd = ml_dtypes.bfloat16
    x = np.asarray(x, np.float32)
    Wq = np.asarray(Wq, np.float32); Wk = np.asarray(Wk, np.float32)
    Wv = np.asarray(Wv, np.float32); Wo = np.asarray(Wo, np.float32)
    bq = np.asarray(bq, np.float32); bk = np.asarray(bk, np.float32)
    temp = np.asarray(temperature, np.float32).reshape(-1)
    mask = np.asarray(sparse_mask)

    in_maps = []
    for c in CORE_IDS:
        b, g = c // 2, c % 2
        cols = slice(g * LOC, (g + 1) * LOC)
        hs = slice(g * GH, (g + 1) * GH)
        xt_h = np.ascontiguousarray(
            x[b].T.reshape(8, 128, 1024).transpose(1, 0, 2)).astype(bfd)
        wq_h = np.ascontiguousarray(
            Wq[:, cols].reshape(8, 128, 4, 128).transpose(1, 2, 0, 3)).astype(bfd)
        wk_h = np.ascontiguousarray(
            Wk[:, cols].reshape(8, 128, 4, 128).transpose(1, 2, 0, 3)).astype(bfd)
        wv_h = np.ascontiguousarray(
            Wv[:, cols].reshape(8, 128, 512).transpose(1, 0, 2)).astype(bfd)
        wo_h = np.ascontiguousarray(
            Wo[cols, :].reshape(4, 128, 1024).transpose(1, 0, 2)).astype(bfd)
        # mask pairs: [4 j, 128 p, 8 kt, 2 h, 1024 q]; element (j,p,kt,h,q) =
        # sparse_mask[b, hs[2j+h], q, kt*128+p]
        mt = mask[b, hs].transpose(0, 2, 1)            # [8h, 1024k, 1024q]
        mp_h = np.ascontiguousarray(
            mt.reshape(4, 2, 8, 128, 1024).transpose(0, 3, 2, 1, 4)
        ).astype(bfd).reshape(4, 128, 16384)
        in_maps.append({
            "xt": xt_h, "wq": wq_h, "wk": wk_h, "wv": wv_h, "wo": wo_h,
            "mp": mp_h,
            "tempx": (np.repeat(temp[hs], D) / np.sqrt(D)).astype(np.float32),
            "bqv": np.ascontiguousarray(bq[cols]),
            "bkv": np.ascontiguousarray(bk[cols]),
        })
    return in_maps


def kernel(**inputs):
    in_maps = _prep_inputs(**inputs)
    nc = _get_nc()
    res = run_bass_kernel_spmd(nc, in_maps, CORE_IDS)
    # unshard: row-parallel partial sum per batch + constant bias row
    # (softmax rows sum to 1 so bv contributes bv @ Wo to every row)
    bv = np.asarray(inputs["bv"], np.float32)
    bo = np.asarray(inputs["bo"], np.float32)
    Wo = np.asarray(inputs["Wo"], np.float32)
    brow = bv @ Wo + bo
    out = np.empty((B, S, HID), np.float32)
    for b in range(B):
        out[b] = (res.results[2 * b]["out"].astype(np.float32)
                  + res.results[2 * b + 1]["out"].astype(np.float32) + brow)
    return out


# revision 60
# speedup vs baseline: 1.0860x; 1.0106x over previous
"""Trainium2 Bass kernel for nn_AdaptiveAttention (sparse attention, B=4 S=1024 HID=1024 H=16).

Sharding (8 cores): core c = (batch b=c//2) x (head-group g=c%2, 8 heads / 512 hid cols).

v2 design (cost-model driven; ~121.7us vs 151.9us v1 baseline):
- All DRAM inputs host-pre-tiled into exact SBUF layouts so every DMA is a
  contiguous >=1KB-run burst (full-rate in the DMA model; elem runs >=512B).
- Q^T/K^T = W x x^T with temperature/sqrt(D) folded into the Q eviction
  (DVE tensor_scalar).
- Attention runs in 16 quarter-windows (j head-pair x qc 256-q columns):
  scores are kt-PAIRED into one [128,1024] PSUM tile per two k-tiles: the A
  (tile_position (0,0)) halves fill bank X (cols 0:512), the B ((64,0))
  halves bank Y (cols 512:1024) -- a tile_position pair sharing a bank, or
  any start at a non-bank-aligned offset, crashes the hw, but one group per
  bank (start only on the even kt, lazy zero-region covering the odd kt's
  cols) is legal. One exp (ACT) and one mask-multiply (DVE 2x bf16) then
  cover 2 heads x 2 k-tiles, halving ACT op count (64 exps total).
- AV restructured: stationary = P^T tile [128k,128q], moving = Vext [128k,65]
  (ones column) -> av [128q,65] accumulated in one bank-sized PSUM tile as a
  SINGLE accumulation group (start only on the first matmul: the lazy
  zero-region covers all 4 chains; stop only on the last). Halves AV
  tensor-engine rows (ap=65 vs 512; LdWeights is free in the cost model) and
  makes the softmax denominator a per-PARTITION column: normalization is one
  reciprocal + one broadcast multiply -- no DMA broadcasts at all.
- Each window's AV matmuls run one FULL window later (pt tiles are a window
  old, so the exp->mask chain can never stall them); att[q,d] returns to
  attT[d,q] via PE transposes (4/window) batched in a dedicated PSUM bank
  with one [128,256] eviction into attl.
- Junk warm-up matmuls ramp the PE p-state during the DMA prologue; V-chunk
  evictions alternate ACT/DVE; out-projection chunks evict to f16 (halves
  output DMA) alternating ACT/DVE, host sums partials + (bv@Wo+bo) row.
- DMA queue discipline matters as much as bytes: the gpsimd/SWDGE queue
  carries ONLY the mask streams (its per-DMA ~1us Pool desc-gen would
  otherwise delay mask kt-tiles and head-of-line-block window 0's mask
  multiplies -> DVE FIFO -> eviction ring -> PE); the 3 tiny consts ride the
  sync/HWDGE queue ahead of weights, and wv precedes the j1-3 W slices so
  the V chunks are never input-gated.
- PSUM = exactly 8 banks: scores 2x[128,1024] + av [128,512] + transpose
  batch [128,1024]bf16 + 2x[128,512] general ring (projections/V/out-proj).
"""
import os
import sys

for _p in ("/opt/trn_rl_repo", "/root/.axon_site/_ro/trn_rl_repo"):
    if os.path.isdir(_p) and _p not in sys.path:
        sys.path.insert(0, _p)

import numpy as np
import ml_dtypes

import concourse.bass as bass
from concourse import bacc
import concourse.mybir as mybir
import concourse.tile as tile
from concourse.bass_utils import run_bass_kernel_spmd
from concourse.masks import make_identity

B, S, HID, H, D = 4, 1024, 1024, 16, 64
NCORES = 8
GH = 8          # heads per core
LOC = GH * D    # 512, local hid slice
CORE_IDS = list(range(NCORES))

bf16 = mybir.dt.bfloat16
f32 = mybir.dt.float32
AF = mybir.ActivationFunctionType
ALU = mybir.AluOpType

_NC_CACHE = None


def _build(dbg=False, reps=1):
    nc = bacc.Bacc("TRN2", debug=False, num_devices=NCORES)

    xt = nc.declare_dram_parameter("xt", [128, 8, 1024], bf16, False)
    wq = nc.declare_dram_parameter("wq", [128, 4, 8, 128], bf16, False)
    wk = nc.declare_dram_parameter("wk", [128, 4, 8, 128], bf16, False)
    wv = nc.declare_dram_parameter("wv", [128, 8, 512], bf16, False)
    wo = nc.declare_dram_parameter("wo", [128, 4, 1024], bf16, False)
    mp = nc.declare_dram_parameter("mp", [4, 128, 8 * 2 * 1024], bf16, False)
    cst = nc.declare_dram_parameter("cst", [128, 12], f32, False)
    f16 = mybir.dt.float16
    out = nc.declare_dram_parameter("out", [S, HID], f16, True)    # partial

    with tile.TileContext(nc) as tc:
        with (
            tc.tile_pool(name="pw", bufs=4) as pw,           # weights
            tc.tile_pool(name="pxt", bufs=1) as pxt,         # xT [128,8192] bf16
            tc.tile_pool(name="pqk", bufs=8) as pqk,         # QT/KT [128,1024] bf16
            tc.tile_pool(name="pv", bufs=8) as pv,           # Vext [128,520] bf16
            tc.tile_pool(name="ppt", bufs=10) as ppt,        # P^T kt-pair [128,1024] bf16
            tc.tile_pool(name="pmask", bufs=2) as pmask,     # mask pair [128,16384] bf16
            tc.tile_pool(name="patt", bufs=4) as patt,       # att [128,256] bf16
            tc.tile_pool(name="pattl", bufs=4) as pattl,     # attT per j (live to end)
            tc.tile_pool(name="prcp", bufs=3) as prcp,       # recip [128,4] f32
            tc.tile_pool(name="pout", bufs=4) as pout,       # out staging f16
            tc.tile_pool(name="pconst", bufs=1) as pconst,   # small tiles
            tc.tile_pool(name="psc", bufs=2, space="PSUM") as psc,   # scores [128,1024]
            tc.tile_pool(name="pav", bufs=1, space="PSUM") as pav,   # av [128,512]
            tc.tile_pool(name="ptrp", bufs=1, space="PSUM") as ptrp, # transposes
            tc.tile_pool(name="pps", bufs=2, space="PSUM") as pps,   # general ring
        ):
            for _rep in range(reps):
                # ---- small constants (gpsimd queue, ahead of masks) ----
                # warm-up memset first: no deps, so the PE warm-up isn't
                # queued behind the const DMAs on DVE
                wrm = pconst.tile([128, 512], bf16, tag="wrm")
                nc.vector.memset(wrm[:], 0.0)

                # identity for PE transposes
                ident = pconst.tile([128, 128], bf16, tag="ident")
                make_identity(nc, ident[:])

                # ---- PE warm-up: junk matmuls ramp the p-state while DMAs
                # stream; their PSUM tile is write-only and recycled ----
                wps = pps.tile([128, 512], f32, tag="pp", name="warmps")
                for _w in range(20):
                    nc.tensor.matmul(wps[:], wrm[:, 0:128], wrm[:],
                                     start=True, stop=True)

                # ---- bulk loads (sync queue = HWDGE) ----
                wqb = pw.tile([128, 4 * 8 * 128], bf16, tag="w", name="wq")
                wq4 = wqb[:].rearrange("p (j c n) -> p j c n", j=4, c=8)
                wkb = pw.tile([128, 4 * 8 * 128], bf16, tag="w", name="wk")
                wk4 = wkb[:].rearrange("p (j c n) -> p j c n", j=4, c=8)
                xtb = pxt.tile([128, 8 * 1024], bf16, tag="xt", name="xt")
                xt3 = xtb[:].rearrange("p (c s) -> p c s", c=8)

                cst_t = pconst.tile([128, 12], f32, tag="cst", name="cst")
                nc.sync.dma_start(out=cst_t[:], in_=cst[:])
                scale_t = cst_t[:, 0:4]
                bq_t = cst_t[:, 4:8]
                bk_t = cst_t[:, 8:12]
                bqs_t = pconst.tile([128, 4], f32, tag="bqs")
                nc.vector.tensor_mul(bqs_t[:], bq_t, scale_t)
                # preload exp table during the DMA phase
                warm_t = pconst.tile([1, 4], f32, tag="warmexp")
                nc.scalar.activation(warm_t[:], scale_t[0:1, :], AF.Exp)
                nc.sync.dma_start(out=wq4[:, 0], in_=wq[:, 0])
                nc.sync.dma_start(out=xt3[:, 0:4, 0:512], in_=xt[:, 0:4, 0:512])
                nc.sync.dma_start(out=xt3[:, 4:8, 0:512], in_=xt[:, 4:8, 0:512])
                nc.sync.dma_start(out=wk4[:, 0], in_=wk[:, 0])
                nc.sync.dma_start(out=xt3[:, 0:4, 512:1024],
                                  in_=xt[:, 0:4, 512:1024])
                nc.sync.dma_start(out=xt3[:, 4:8, 512:1024],
                                  in_=xt[:, 4:8, 512:1024])
                wvb = pw.tile([128, 8 * 512], bf16, tag="w", name="wv")
                wv3 = wvb[:].rearrange("p (c n) -> p c n", c=8)
                nc.sync.dma_start(out=wv3[:], in_=wv[:])
                for _j in (1, 2, 3):
                    nc.sync.dma_start(out=wq4[:, _j], in_=wq[:, _j])
                for _j in (1, 2, 3):
                    nc.sync.dma_start(out=wk4[:, _j], in_=wk[:, _j])
                wob = pw.tile([128, 4 * 1024], bf16, tag="w", name="wo")
                wo3 = wob[:].rearrange("p (r n) -> p r n", r=4)
                nc.sync.dma_start(out=wo3[:], in_=wo[:])

                # ---- mask pair loads (gpsimd queue), kt-granular ----
                mh = [None] * 4

                def load_mask(j):
                    t = pmask.tile([128, 8 * 2 * 1024], bf16, tag="mask",
                                   name=f"mp{j}")
                    tv = t[:].rearrange("p (k x) -> p k x", k=8)
                    for kt in range(8):
                        nc.gpsimd.dma_start(
                            out=tv[:, kt],
                            in_=mp[j, :, kt * 2048:(kt + 1) * 2048])
                    mh[j] = t

                load_mask(0)
                load_mask(1)

                # ---- Q^T/K^T projection pieces ----
                qtb = [None] * 4
                ktb = [None] * 4

                def qk_alloc(j):
                    qtb[j] = pqk.tile([128, 1024], bf16, tag="qk", name=f"qt{j}")
                    ktb[j] = pqk.tile([128, 1024], bf16, tag="qk", name=f"kt{j}")

                _qk_ps = {}

                # piece 0: Q s-half0, 1: K s-half0, 2: Q s-half1, 3: K s-half1
                def qk_half(j, piece, half, pool=None, ptag="pp", pshape=None):
                    is_q = piece % 2 == 0
                    sh = piece // 2
                    wsrc = wq4 if is_q else wk4
                    key = (j, piece)
                    if half == 0:
                        _qk_ps[key] = (pool or pps).tile(
                            pshape or [128, 512], f32, tag=ptag,
                            name=f"qkps{j}_{piece}")
                    psa = _qk_ps[key][:, 0:512]
                    for c8 in range(4 * half, 4 * half + 4):
                        nc.tensor.matmul(psa, wsrc[:, j, c8, :],
                                         xt3[:, c8, sh * 512:(sh + 1) * 512],
                                         start=(c8 == 0), stop=(c8 == 7))
                    if half == 1:
                        dst = (qtb[j] if is_q else ktb[j])[:, sh * 512:(sh + 1) * 512]
                        if is_q:
                            nc.vector.tensor_scalar(
                                dst, psa, cst_t[:, j:j + 1],
                                bqs_t[:, j:j + 1], ALU.mult, ALU.add)
                        else:
                            nc.vector.tensor_scalar_add(dst, psa,
                                                        cst_t[:, 8 + j:9 + j])

                def qk_piece(j, piece, pool=None, ptag="pp", pshape=None):
                    qk_half(j, piece, 0, pool, ptag, pshape)
                    qk_half(j, piece, 1, pool, ptag, pshape)

                # ---- V projection chunk st -> Vext [128 s, 8*65] ----
                vext = [None] * 8

                def vchunk(st, pool=None, ptag="pp", pshape=None):
                    vps = (pool or pps).tile(pshape or [128, 512], f32,
                                             tag=ptag, name=f"vps{st}")
                    vps = vps[:, 0:512]
                    for c8 in range(8):
                        nc.tensor.matmul(vps, xt3[:, c8, st * 128:(st + 1) * 128],
                                         wv3[:, c8, :], start=(c8 == 0), stop=(c8 == 7))
                    vt = pv.tile([128, 520], bf16, tag="vext", name=f"vext{st}")
                    v3 = vt[:].rearrange("p (h e) -> p h e", e=65)
                    # alternate eviction engine so neither ACT nor DVE piles up
                    if st % 2 == 0:
                        nc.scalar.activation(
                            v3[:, :, 0:64],
                            vps.rearrange("p (h e) -> p h e", e=64), AF.Copy)
                    else:
                        nc.vector.tensor_copy(
                            v3[:, :, 0:64],
                            vps.rearrange("p (h e) -> p h e", e=64))
                    nc.gpsimd.memset(v3[:, :, 64:65], 1.0)
                    vext[st] = vt

                # ---- attention machinery ----
                attl = [None] * 4
                pending_tr = []   # deferred (j, qc, att_tile, widx)

                def flush_transposes():
                    if not pending_tr:
                        return
                    j, qc, att, w = pending_tr.pop(0)
                    ptr_t = ptrp.tile([128, 1024], bf16, tag="ptr",
                                     name=f"ptr{w}")
                    for h in (0, 1):
                        for qt in (0, 1):
                            nc.tensor.transpose(
                                ptr_t[h * 64:(h + 1) * 64,
                                      qt * 128:(qt + 1) * 128],
                                att[:, (h * 2 + qt) * 64:(h * 2 + qt + 1) * 64],
                                ident[:])
                    if attl[j] is None:
                        attl[j] = pattl.tile([128, 1024], bf16, tag="attl",
                                             name=f"attl{j}")
                    nc.vector.tensor_copy(attl[j][:, qc * 256:(qc + 1) * 256],
                                          ptr_t[:, 0:256])

                # pending AV state: the avs of window w run one window later,
                # by which time every pt tile is ~a full window old, so they
                # never wait on the exp/mask chain.
                pend = {}   # pts, j, qc, widx, avt

                def pend_avs(kt):
                    # one kt's worth (4 matmuls) of the previous window's AV
                    if not pend:
                        return
                    if kt == 8:
                        # normalization: per-partition reciprocal+scalar mult
                        avt, pj, pqc, pw = (pend["avt"], pend["j"],
                                            pend["qc"], pend["widx"])
                        att = patt.tile([128, 256], bf16, tag="att",
                                        name=f"att{pw}")
                        rcp = prcp.tile([128, 4], f32, tag="rcp",
                                        name=f"rcp{pw}")
                        avv = avt[:, 0:260].rearrange("p (n e) -> p n e", e=65)
                        nc.vector.reciprocal(rcp[:], avv[:, :, 64])
                        # one broadcast multiply normalizes all 4 chains
                        nc.vector.tensor_mul(
                            att[:].rearrange("p (n e) -> p n e", e=64),
                            avv[:, :, 0:64],
                            rcp[:].to_broadcast([128, 4, 64]))
                        pending_tr.append((pj, pqc, att, pw))
                        pend.clear()
                        return
                    if kt == 0:
                        # single bank-sized av tile, one accumulation group:
                        # the first matmul's start lazily zeroes the whole 2KB
                        # zero region so all 4 chains accumulate from zero.
                        pend["avt"] = pav.tile([128, 512], f32, tag="av",
                                               name=f"av{pend['widx']}")
                    avt = pend["avt"]
                    ptk = pend["pts"][kt // 2]
                    par = kt % 2
                    pj = pend["j"]
                    for h in (0, 1):
                        hh = 2 * pj + h
                        for qt in (0, 1):
                            c = (h * 2 + qt) * 65
                            nc.tensor.matmul(
                                avt[:, c:c + 65],
                                ptk[:, h * 512 + par * 256 + qt * 128:
                                    h * 512 + par * 256 + (qt + 1) * 128],
                                vext[kt][:, hh * 65:(hh + 1) * 65],
                                start=(kt == 0 and h == 0 and qt == 0),
                                stop=(kt == 7 and h == 1 and qt == 1),
                                skip_group_check=True)

                def window(j, qc, widx, fillers):
                    qs = slice(qc * 256, (qc + 1) * 256)
                    # mask view permuted to (h, kt, q) to match pt layout
                    mpv = mh[j][:].rearrange("p (k h q) -> p h k q", k=8, h=2)
                    pts = [None] * 4
                    ps = None
                    for kt in range(9):
                        if kt < 8:
                            # kt-PAIRED scores: one [128,1024] tile per kt
                            # pair; A halves (tile_position (0,0)) in bank X
                            # (cols 0:512), B halves ((64,0)) in bank Y (cols
                            # 512:1024). Only the even kt starts the group --
                            # its lazy zero region covers the odd kt's cols.
                            par = kt % 2
                            if par == 0:
                                ps = psc.tile([128, 1024], f32, tag="sc",
                                              name=f"sc{widx}_{kt // 2}")
                            nc.tensor.matmul(
                                ps[:, par * 256:(par + 1) * 256],
                                ktb[j][0:64, kt * 128:(kt + 1) * 128],
                                qtb[j][0:64, qs],
                                start=(par == 0), stop=(par == 1),
                                tile_position=(0, 0), skip_group_check=True)
                            nc.tensor.matmul(
                                ps[:, 512 + par * 256:512 + (par + 1) * 256],
                                ktb[j][64:128, kt * 128:(kt + 1) * 128],
                                qtb[j][64:128, qs],
                                start=(par == 0), stop=(par == 1),
                                tile_position=(64, 0), skip_group_check=True)
                        pend_avs(kt)
                        for u in fillers[kt]:
                            u()
                        if kt < 8 and kt % 2 == 1:
                            # one exp + one mask multiply per kt PAIR
                            pt = ppt.tile([128, 1024], bf16, tag="pt",
                                          name=f"pt{widx}_{kt // 2}")
                            nc.scalar.activation(pt[:], ps[:], AF.Exp)
                            nc.vector.tensor_mul(
                                pt[:].rearrange("p (h k q) -> p h k q",
                                                h=2, k=2),
                                pt[:].rearrange("p (h k q) -> p h k q",
                                                h=2, k=2),
                                mpv[:, :, kt - 1:kt + 1, qs])
                            pts[kt // 2] = pt
                    pend.update(pts=pts, j=j, qc=qc, widx=widx)

                # ---- out projection chunk (qt, ch): PSUM -> f16 -> DRAM ----
                def outproj(qt, ch):
                    ops = pps.tile([128, 512], f32, tag="pp",
                                   name=f"ops{qt}_{ch}")
                    for rcx in range(4):
                        nc.tensor.matmul(ops[:],
                                         attl[rcx][:, qt * 128:(qt + 1) * 128],
                                         wo3[:, rcx, ch * 512:(ch + 1) * 512],
                                         start=(rcx == 0), stop=(rcx == 3))
                    ot = pout.tile([128, 512], f16, tag="out",
                                   name=f"ot{qt}_{ch}")
                    if ch == 0:
                        nc.vector.tensor_copy(ot[:], ops[:])
                    else:
                        nc.scalar.activation(ot[:], ops[:], AF.Copy)
                    nc.sync.dma_start(
                        out=out[qt * 128:(qt + 1) * 128, ch * 512:(ch + 1) * 512],
                        in_=ot[:])

                # ---- schedule ----
                # head chains spread over the (still idle) psc/pav banks so
                # the 2-deep pps ring doesn't serialize the prologue
                qk_alloc(0)
                qk_piece(0, 0)
                qk_piece(0, 1, psc, "sc", [128, 1024])
                qk_piece(0, 2)
                qk_piece(0, 3, pav, "av")
                vchunk(0)
                vchunk(1)

                _vps = {}

                def vA(st):
                    def g():
                        _vps[st] = pps.tile([128, 512], f32, tag="pp",
                                            name=f"vps{st}")
                        for c8 in range(4):
                            nc.tensor.matmul(
                                _vps[st][:], xt3[:, c8, st * 128:(st + 1) * 128],
                                wv3[:, c8, :], start=(c8 == 0), stop=False)
                    return g

                def vB(st):
                    def g():
                        vps = _vps[st]
                        for c8 in range(4, 8):
                            nc.tensor.matmul(
                                vps[:], xt3[:, c8, st * 128:(st + 1) * 128],
                                wv3[:, c8, :], start=False, stop=(c8 == 7))
                        vt = pv.tile([128, 520], bf16, tag="vext",
                                     name=f"vext{st}")
                        v3 = vt[:].rearrange("p (h e) -> p h e", e=65)
                        if st % 2 == 0:
                            nc.scalar.activation(
                                v3[:, :, 0:64],
                                vps[:].rearrange("p (h e) -> p h e", e=64),
                                AF.Copy)
                        else:
                            nc.vector.tensor_copy(
                                v3[:, :, 0:64],
                                vps[:].rearrange("p (h e) -> p h e", e=64))
                        nc.gpsimd.memset(v3[:, :, 64:65], 1.0)
                        vext[st] = vt
                    return g

                def qkh(j, piece, half):
                    def g():
                        if piece == 0 and half == 0:
                            qk_alloc(j)
                        qk_half(j, piece, half)
                    return g

                def op(qt, ch):
                    return lambda: outproj(qt, ch)

                T = flush_transposes

                def slots(*units_at):
                    # units_at: dict slot -> list of units
                    f = [[] for _ in range(9)]
                    for s, us in units_at:
                        f[s].extend(us)
                    return f

                # per-window filler plans. Constraints: vext[kt] ready before
                # window 1 slot kt (all vchunks inside window 0); Q/K sh0 of
                # pair j evicted before window (j,0), K sh1 before its slot 4,
                # Q sh1 before (j,2); out-proj chunks once their attl quarter
                # (evicted 2 windows after the producing window) exists.
                F = [
                    slots((0, [vA(2)]), (1, [vB(2)]), (2, [vA(3)]),
                          (3, [vB(3)]), (4, [vA(4)]), (5, [vB(4)]),
                          (6, [vA(5)]), (7, [vB(5)])),
                    slots((0, [T]), (1, [vA(6)]), (2, [vB(6)]),
                          (3, [vA(7)]), (4, [vB(7)]),
                          (5, [qkh(1, 0, 0)]), (7, [qkh(1, 0, 1)])),
                    slots((0, [T]), (1, [qkh(1, 1, 0)]), (4, [qkh(1, 1, 1)])),
                    slots((0, [T]), (1, [qkh(1, 3, 0)]), (4, [qkh(1, 3, 1)])),
                    slots((0, [T]), (1, [qkh(1, 2, 0)]), (4, [qkh(1, 2, 1)])),
                    slots((0, [T]), (1, [qkh(2, 0, 0)]), (4, [qkh(2, 0, 1)])),
                    slots((0, [T]), (1, [qkh(2, 1, 0)]), (4, [qkh(2, 1, 1)])),
                    slots((0, [T]), (1, [qkh(2, 3, 0)]), (4, [qkh(2, 3, 1)])),
                    slots((0, [T]), (1, [qkh(2, 2, 0)]), (4, [qkh(2, 2, 1)])),
                    slots((0, [T]), (1, [qkh(3, 0, 0)]), (4, [qkh(3, 0, 1)])),
                    slots((0, [T]), (1, [qkh(3, 1, 0)]), (4, [qkh(3, 1, 1)])),
                    slots((0, [T]), (1, [qkh(3, 3, 0)]), (4, [qkh(3, 3, 1)])),
                    slots((0, [T]), (1, [qkh(3, 2, 0)]), (4, [qkh(3, 2, 1)])),
                    slots((0, [T])),
                    slots((0, [T]), (1, [op(0, 0)]), (3, [op(0, 1)]),
                          (5, [op(1, 0)]), (7, [op(1, 1)])),
                    slots((0, [T]), (1, [op(2, 0)]), (3, [op(2, 1)]),
                          (5, [op(3, 0)]), (7, [op(3, 1)])),
                ]

                widx = 0
                for j in range(4):
                    for qc in range(4):
                        window(j, qc, widx, F[widx])
                        widx += 1
                        if (j, qc) == (0, 1):
                            load_mask(2)
                        elif (j, qc) == (1, 1):
                            load_mask(3)

                # tail: flush window 14's transposes (enables qt 4/5 chunks),
                # run the final window's AVs interleaved with those chunks,
                # then its norm + transposes and the last out-proj chunks.
                flush_transposes()   # att of window 14 -> attl[3] cols 512:768
                tail_ops = [op(4, 0), op(4, 1), op(5, 0), op(5, 1)]
                for kt in range(9):
                    pend_avs(kt)
                    if kt in (1, 3, 5, 7):
                        tail_ops.pop(0)()
                # open the last four chunks (rcx 0-2 need only attl[0..2])
                # before the final transpose flush; close after it
                t_ops = []
                for qt, ch in ((6, 0), (6, 1), (7, 0), (7, 1)):
                    ops = pps.tile([128, 512], f32, tag="pp",
                                   name=f"ops{qt}_{ch}")
                    for rcx in range(3):
                        nc.tensor.matmul(
                            ops[:], attl[rcx][:, qt * 128:(qt + 1) * 128],
                            wo3[:, rcx, ch * 512:(ch + 1) * 512],
                            start=(rcx == 0), stop=False)
                    t_ops.append((qt, ch, ops))
                    if (qt, ch) == (6, 1):
                        flush_transposes()   # att of w15 -> attl[3] cols 768:
                for qt, ch, ops in t_ops:
                    nc.tensor.matmul(ops[:],
                                     attl[3][:, qt * 128:(qt + 1) * 128],
                                     wo3[:, 3, ch * 512:(ch + 1) * 512],
                                     start=False, stop=True)
                    ot = pout.tile([128, 512], f16, tag="out",
                                   name=f"otf{qt}_{ch}")
                    if ch == 0:
                        nc.vector.tensor_copy(ot[:], ops[:])
                    else:
                        nc.scalar.activation(ot[:], ops[:], AF.Copy)
                    nc.sync.dma_start(
                        out=out[qt * 128:(qt + 1) * 128,
                                ch * 512:(ch + 1) * 512],
                        in_=ot[:])

    nc.compile()
    return nc


def _get_nc():
    global _NC_CACHE
    if _NC_CACHE is None:
        _NC_CACHE = _build()
    return _NC_CACHE


def _prep_inputs(x, Wq, bq, Wk, bk, Wv, bv, Wo, bo, temperature, sparse_mask):
    bfd = ml_dtypes.bfloat16
    x = np.asarray(x, np.float32)
    Wq = np.asarray(Wq, np.float32); Wk = np.asarray(Wk, np.float32)
    Wv = np.asarray(Wv, np.float32); Wo = np.asarray(Wo, np.float32)
    bq = np.asarray(bq, np.float32); bk = np.asarray(bk, np.float32)
    temp = np.asarray(temperature, np.float32).reshape(-1)
    mask = np.asarray(sparse_mask)

    in_maps = []
    for c in CORE_IDS:
        b, g = c // 2, c % 2
        cols = slice(g * LOC, (g + 1) * LOC)
        hs = slice(g * GH, (g + 1) * GH)
        xt_h = np.ascontiguousarray(
            x[b].T.reshape(8, 128, 1024).transpose(1, 0, 2)).astype(bfd)
        wq_h = np.ascontiguousarray(
            Wq[:, cols].reshape(8, 128, 4, 128).transpose(1, 2, 0, 3)).astype(bfd)
        wk_h = np.ascontiguousarray(
            Wk[:, cols].reshape(8, 128, 4, 128).transpose(1, 2, 0, 3)).astype(bfd)
        wv_h = np.ascontiguousarray(
            Wv[:, cols].reshape(8, 128, 512).transpose(1, 0, 2)).astype(bfd)
        wo_h = np.ascontiguousarray(
            Wo[cols, :].reshape(4, 128, 1024).transpose(1, 0, 2)).astype(bfd)
        # mask pairs: [4 j, 128 p, 8 kt, 2 h, 1024 q]; element (j,p,kt,h,q) =
        # sparse_mask[b, hs[2j+h], q, kt*128+p]
        mt = mask[b, hs].transpose(0, 2, 1)            # [8h, 1024k, 1024q]
        mp_h = np.ascontiguousarray(
            mt.reshape(4, 2, 8, 128, 1024).transpose(0, 3, 2, 1, 4)
        ).astype(bfd).reshape(4, 128, 16384)
        cst_h = np.stack([
            (np.repeat(temp[hs], D) / np.sqrt(D)).astype(np.float32),
            bq[cols].astype(np.float32),
            bk[cols].astype(np.float32),
        ]).reshape(3, 4, 128).transpose(2, 0, 1).reshape(128, 12)
        in_maps.append({
            "xt": xt_h, "wq": wq_h, "wk": wk_h, "wv": wv_h, "wo": wo_h,
            "mp": mp_h, "cst": np.ascontiguousarray(cst_h),
        })
    return in_maps


def kernel(**inputs):
    in_maps = _prep_inputs(**inputs)
    nc = _get_nc()
    res = run_bass_kernel_spmd(nc, in_maps, CORE_IDS)
    # unshard: row-parallel partial sum per batch + constant bias row
    # (softmax rows sum to 1 so bv contributes bv @ Wo to every row)
    bv = np.asarray(inputs["bv"], np.float32)
    bo = np.asarray(inputs["bo"], np.float32)
    Wo = np.asarray(inputs["Wo"], np.float32)
    brow = bv @ Wo + bo
    out = np.empty((B, S, HID), np.float32)
    for b in range(B):
        out[b] = (res.results[2 * b]["out"].astype(np.float32)
                  + res.results[2 * b + 1]["out"].astype(np.float32) + brow)
    return out


# revision 61
# speedup vs baseline: 1.0909x; 1.0045x over previous
"""Trainium2 Bass kernel for nn_AdaptiveAttention (sparse attention, B=4 S=1024 HID=1024 H=16).

Sharding (8 cores): core c = (batch b=c//2) x (head-group g=c%2, 8 heads / 512 hid cols).

v2 design (cost-model driven; ~121.7us vs 151.9us v1 baseline):
- All DRAM inputs host-pre-tiled into exact SBUF layouts so every DMA is a
  contiguous >=1KB-run burst (full-rate in the DMA model; elem runs >=512B).
- Q^T/K^T = W x x^T with temperature/sqrt(D) folded into the Q eviction
  (DVE tensor_scalar).
- Attention runs in 16 quarter-windows (j head-pair x qc 256-q columns):
  scores are kt-PAIRED into one [128,1024] PSUM tile per two k-tiles: the A
  (tile_position (0,0)) halves fill bank X (cols 0:512), the B ((64,0))
  halves bank Y (cols 512:1024) -- a tile_position pair sharing a bank, or
  any start at a non-bank-aligned offset, crashes the hw, but one group per
  bank (start only on the even kt, lazy zero-region covering the odd kt's
  cols) is legal. One exp (ACT) and one mask-multiply (DVE 2x bf16) then
  cover 2 heads x 2 k-tiles, halving ACT op count (64 exps total).
- AV restructured: stationary = P^T tile [128k,128q], moving = Vext [128k,65]
  (ones column) -> av [128q,65] accumulated in one bank-sized PSUM tile as a
  SINGLE accumulation group (start only on the first matmul: the lazy
  zero-region covers all 4 chains; stop only on the last). Halves AV
  tensor-engine rows (ap=65 vs 512; LdWeights is free in the cost model) and
  makes the softmax denominator a per-PARTITION column: normalization is one
  reciprocal + one broadcast multiply -- no DMA broadcasts at all.
- Each window's AV matmuls run one FULL window later (pt tiles are a window
  old, so the exp->mask chain can never stall them); att[q,d] returns to
  attT[d,q] via PE transposes (4/window) batched in a dedicated PSUM bank
  with one [128,256] eviction into attl.
- Junk warm-up matmuls ramp the PE p-state during the DMA prologue; V-chunk
  evictions alternate ACT/DVE; out-projection chunks evict to f16 (halves
  output DMA) alternating ACT/DVE, host sums partials + (bv@Wo+bo) row.
- DMA queue discipline matters as much as bytes: the gpsimd/SWDGE queue
  carries ONLY the mask streams (its per-DMA ~1us Pool desc-gen would
  otherwise delay mask kt-tiles and head-of-line-block window 0's mask
  multiplies -> DVE FIFO -> eviction ring -> PE); the 3 tiny consts ride the
  sync/HWDGE queue ahead of weights, and wv precedes the j1-3 W slices so
  the V chunks are never input-gated.
- PSUM = exactly 8 banks: scores 2x[128,1024] + av [128,512] + transpose
  batch [128,1024]bf16 + 2x[128,512] general ring (projections/V/out-proj).
"""
import os
import sys

for _p in ("/opt/trn_rl_repo", "/root/.axon_site/_ro/trn_rl_repo"):
    if os.path.isdir(_p) and _p not in sys.path:
        sys.path.insert(0, _p)

import numpy as np
import ml_dtypes

import concourse.bass as bass
from concourse import bacc
import concourse.mybir as mybir
import concourse.tile as tile
from concourse.bass_utils import run_bass_kernel_spmd
from concourse.masks import make_identity

B, S, HID, H, D = 4, 1024, 1024, 16, 64
NCORES = 8
GH = 8          # heads per core
LOC = GH * D    # 512, local hid slice
CORE_IDS = list(range(NCORES))

bf16 = mybir.dt.bfloat16
f32 = mybir.dt.float32
AF = mybir.ActivationFunctionType
ALU = mybir.AluOpType

_NC_CACHE = None


def _build(dbg=False, reps=1):
    nc = bacc.Bacc("TRN2", debug=False, num_devices=NCORES)

    xt = nc.declare_dram_parameter("xt", [128, 8, 1024], bf16, False)
    wq = nc.declare_dram_parameter("wq", [128, 4, 8, 128], bf16, False)
    wk = nc.declare_dram_parameter("wk", [128, 4, 8, 128], bf16, False)
    wv = nc.declare_dram_parameter("wv", [128, 8, 512], bf16, False)
    wo = nc.declare_dram_parameter("wo", [128, 4, 1024], bf16, False)
    mp = nc.declare_dram_parameter("mp", [4, 128, 8 * 2 * 1024], bf16, False)
    cst = nc.declare_dram_parameter("cst", [128, 12], f32, False)
    f16 = mybir.dt.float16
    out = nc.declare_dram_parameter("out", [S, HID], f16, True)    # partial

    with tile.TileContext(nc) as tc:
        with (
            tc.tile_pool(name="pw", bufs=4) as pw,           # weights
            tc.tile_pool(name="pxt", bufs=1) as pxt,         # xT [128,8192] bf16
            tc.tile_pool(name="pqk", bufs=8) as pqk,         # QT/KT [128,1024] bf16
            tc.tile_pool(name="pv", bufs=8) as pv,           # Vext [128,520] bf16
            tc.tile_pool(name="ppt", bufs=10) as ppt,        # P^T kt-pair [128,1024] bf16
            tc.tile_pool(name="pmask", bufs=2) as pmask,     # mask pair [128,16384] bf16
            tc.tile_pool(name="patt", bufs=4) as patt,       # att [128,256] bf16
            tc.tile_pool(name="pattl", bufs=4) as pattl,     # attT per j (live to end)
            tc.tile_pool(name="prcp", bufs=3) as prcp,       # recip [128,4] f32
            tc.tile_pool(name="pout", bufs=4) as pout,       # out staging f16
            tc.tile_pool(name="pconst", bufs=1) as pconst,   # small tiles
            tc.tile_pool(name="psc", bufs=2, space="PSUM") as psc,   # scores [128,1024]
            tc.tile_pool(name="pav", bufs=1, space="PSUM") as pav,   # av [128,512]
            tc.tile_pool(name="ptrp", bufs=1, space="PSUM") as ptrp, # transposes
            tc.tile_pool(name="pps", bufs=2, space="PSUM") as pps,   # general ring
        ):
            for _rep in range(reps):
                # ---- small constants (gpsimd queue, ahead of masks) ----
                # warm-up memset first: no deps, so the PE warm-up isn't
                # queued behind the const DMAs on DVE
                wrm = pconst.tile([128, 128], bf16, tag="wrm")
                nc.vector.memset(wrm[:], 0.0)

                # identity for PE transposes
                ident = pconst.tile([128, 128], bf16, tag="ident")
                make_identity(nc, ident[:])

                # ---- PE warm-up: junk matmuls ramp the p-state while DMAs
                # stream; their PSUM tile is write-only and recycled ----
                wps = pps.tile([128, 512], f32, tag="pp", name="warmps")
                for _w in range(48):
                    nc.tensor.matmul(wps[:, 0:128], wrm[:], wrm[:],
                                     start=True, stop=True)

                # ---- bulk loads (sync queue = HWDGE) ----
                wqb = pw.tile([128, 4 * 8 * 128], bf16, tag="w", name="wq")
                wq4 = wqb[:].rearrange("p (j c n) -> p j c n", j=4, c=8)
                wkb = pw.tile([128, 4 * 8 * 128], bf16, tag="w", name="wk")
                wk4 = wkb[:].rearrange("p (j c n) -> p j c n", j=4, c=8)
                xtb = pxt.tile([128, 8 * 1024], bf16, tag="xt", name="xt")
                xt3 = xtb[:].rearrange("p (c s) -> p c s", c=8)

                cst_t = pconst.tile([128, 12], f32, tag="cst", name="cst")
                nc.sync.dma_start(out=cst_t[:], in_=cst[:])
                scale_t = cst_t[:, 0:4]
                bq_t = cst_t[:, 4:8]
                bk_t = cst_t[:, 8:12]
                bqs_t = pconst.tile([128, 4], f32, tag="bqs")
                nc.vector.tensor_mul(bqs_t[:], bq_t, scale_t)
                # preload exp table during the DMA phase
                warm_t = pconst.tile([1, 4], f32, tag="warmexp")
                nc.scalar.activation(warm_t[:], scale_t[0:1, :], AF.Exp)
                nc.sync.dma_start(out=wq4[:, 0], in_=wq[:, 0])
                nc.sync.dma_start(out=xt3[:, 0:4, 0:512], in_=xt[:, 0:4, 0:512])
                nc.sync.dma_start(out=xt3[:, 4:8, 0:512], in_=xt[:, 4:8, 0:512])
                nc.sync.dma_start(out=wk4[:, 0], in_=wk[:, 0])
                nc.sync.dma_start(out=xt3[:, 0:4, 512:1024],
                                  in_=xt[:, 0:4, 512:1024])
                nc.sync.dma_start(out=xt3[:, 4:8, 512:1024],
                                  in_=xt[:, 4:8, 512:1024])
                wvb = pw.tile([128, 8 * 512], bf16, tag="w", name="wv")
                wv3 = wvb[:].rearrange("p (c n) -> p c n", c=8)
                nc.sync.dma_start(out=wv3[:], in_=wv[:])
                for _j in (1, 2, 3):
                    nc.sync.dma_start(out=wq4[:, _j], in_=wq[:, _j])
                for _j in (1, 2, 3):
                    nc.sync.dma_start(out=wk4[:, _j], in_=wk[:, _j])
                wob = pw.tile([128, 4 * 1024], bf16, tag="w", name="wo")
                wo3 = wob[:].rearrange("p (r n) -> p r n", r=4)
                nc.sync.dma_start(out=wo3[:], in_=wo[:])

                # ---- mask pair loads (gpsimd queue), kt-granular ----
                mh = [None] * 4

                def load_mask(j):
                    t = pmask.tile([128, 8 * 2 * 1024], bf16, tag="mask",
                                   name=f"mp{j}")
                    tv = t[:].rearrange("p (k x) -> p k x", k=8)
                    for kt in range(8):
                        nc.gpsimd.dma_start(
                            out=tv[:, kt],
                            in_=mp[j, :, kt * 2048:(kt + 1) * 2048])
                    mh[j] = t

                load_mask(0)
                load_mask(1)

                # ---- Q^T/K^T projection pieces ----
                qtb = [None] * 4
                ktb = [None] * 4

                def qk_alloc(j):
                    qtb[j] = pqk.tile([128, 1024], bf16, tag="qk", name=f"qt{j}")
                    ktb[j] = pqk.tile([128, 1024], bf16, tag="qk", name=f"kt{j}")

                _qk_ps = {}

                # piece 0: Q s-half0, 1: K s-half0, 2: Q s-half1, 3: K s-half1
                def qk_half(j, piece, half, pool=None, ptag="pp", pshape=None):
                    is_q = piece % 2 == 0
                    sh = piece // 2
                    wsrc = wq4 if is_q else wk4
                    key = (j, piece)
                    if half == 0:
                        _qk_ps[key] = (pool or pps).tile(
                            pshape or [128, 512], f32, tag=ptag,
                            name=f"qkps{j}_{piece}")
                    psa = _qk_ps[key][:, 0:512]
                    for c8 in range(4 * half, 4 * half + 4):
                        nc.tensor.matmul(psa, wsrc[:, j, c8, :],
                                         xt3[:, c8, sh * 512:(sh + 1) * 512],
                                         start=(c8 == 0), stop=(c8 == 7))
                    if half == 1:
                        dst = (qtb[j] if is_q else ktb[j])[:, sh * 512:(sh + 1) * 512]
                        if is_q:
                            nc.vector.tensor_scalar(
                                dst, psa, cst_t[:, j:j + 1],
                                bqs_t[:, j:j + 1], ALU.mult, ALU.add)
                        else:
                            nc.vector.tensor_scalar_add(dst, psa,
                                                        cst_t[:, 8 + j:9 + j])

                def qk_piece(j, piece, pool=None, ptag="pp", pshape=None):
                    qk_half(j, piece, 0, pool, ptag, pshape)
                    qk_half(j, piece, 1, pool, ptag, pshape)

                # ---- V projection chunk st -> Vext [128 s, 8*65] ----
                vext = [None] * 8

                def vchunk(st, pool=None, ptag="pp", pshape=None):
                    vps = (pool or pps).tile(pshape or [128, 512], f32,
                                             tag=ptag, name=f"vps{st}")
                    vps = vps[:, 0:512]
                    for c8 in range(8):
                        nc.tensor.matmul(vps, xt3[:, c8, st * 128:(st + 1) * 128],
                                         wv3[:, c8, :], start=(c8 == 0), stop=(c8 == 7))
                    vt = pv.tile([128, 520], bf16, tag="vext", name=f"vext{st}")
                    v3 = vt[:].rearrange("p (h e) -> p h e", e=65)
                    # alternate eviction engine so neither ACT nor DVE piles up
                    if st % 2 == 0:
                        nc.scalar.activation(
                            v3[:, :, 0:64],
                            vps.rearrange("p (h e) -> p h e", e=64), AF.Copy)
                    else:
                        nc.vector.tensor_copy(
                            v3[:, :, 0:64],
                            vps.rearrange("p (h e) -> p h e", e=64))
                    nc.gpsimd.memset(v3[:, :, 64:65], 1.0)
                    vext[st] = vt

                # ---- attention machinery ----
                attl = [None] * 4
                pending_tr = []   # deferred (j, qc, att_tile, widx)

                def flush_transposes():
                    if not pending_tr:
                        return
                    j, qc, att, w = pending_tr.pop(0)
                    ptr_t = ptrp.tile([128, 1024], bf16, tag="ptr",
                                     name=f"ptr{w}")
                    for h in (0, 1):
                        for qt in (0, 1):
                            nc.tensor.transpose(
                                ptr_t[h * 64:(h + 1) * 64,
                                      qt * 128:(qt + 1) * 128],
                                att[:, (h * 2 + qt) * 64:(h * 2 + qt + 1) * 64],
                                ident[:])
                    if attl[j] is None:
                        attl[j] = pattl.tile([128, 1024], bf16, tag="attl",
                                             name=f"attl{j}")
                    nc.vector.tensor_copy(attl[j][:, qc * 256:(qc + 1) * 256],
                                          ptr_t[:, 0:256])

                # pending AV state: the avs of window w run one window later,
                # by which time every pt tile is ~a full window old, so they
                # never wait on the exp/mask chain.
                pend = {}   # pts, j, qc, widx, avt

                def pend_avs(kt):
                    # one kt's worth (4 matmuls) of the previous window's AV
                    if not pend:
                        return
                    if kt == 8:
                        # normalization: per-partition reciprocal+scalar mult
                        avt, pj, pqc, pw = (pend["avt"], pend["j"],
                                            pend["qc"], pend["widx"])
                        att = patt.tile([128, 256], bf16, tag="att",
                                        name=f"att{pw}")
                        rcp = prcp.tile([128, 4], f32, tag="rcp",
                                        name=f"rcp{pw}")
                        avv = avt[:, 0:260].rearrange("p (n e) -> p n e", e=65)
                        nc.vector.reciprocal(rcp[:], avv[:, :, 64])
                        # one broadcast multiply normalizes all 4 chains
                        nc.vector.tensor_mul(
                            att[:].rearrange("p (n e) -> p n e", e=64),
                            avv[:, :, 0:64],
                            rcp[:].to_broadcast([128, 4, 64]))
                        pending_tr.append((pj, pqc, att, pw))
                        pend.clear()
                        return
                    if kt == 0:
                        # single bank-sized av tile, one accumulation group:
                        # the first matmul's start lazily zeroes the whole 2KB
                        # zero region so all 4 chains accumulate from zero.
                        pend["avt"] = pav.tile([128, 512], f32, tag="av",
                                               name=f"av{pend['widx']}")
                    avt = pend["avt"]
                    ptk = pend["pts"][kt // 2]
                    par = kt % 2
                    pj = pend["j"]
                    for h in (0, 1):
                        hh = 2 * pj + h
                        for qt in (0, 1):
                            c = (h * 2 + qt) * 65
                            nc.tensor.matmul(
                                avt[:, c:c + 65],
                                ptk[:, h * 512 + par * 256 + qt * 128:
                                    h * 512 + par * 256 + (qt + 1) * 128],
                                vext[kt][:, hh * 65:(hh + 1) * 65],
                                start=(kt == 0 and h == 0 and qt == 0),
                                stop=(kt == 7 and h == 1 and qt == 1),
                                skip_group_check=True)

                def window(j, qc, widx, fillers):
                    qs = slice(qc * 256, (qc + 1) * 256)
                    # mask view permuted to (h, kt, q) to match pt layout
                    mpv = mh[j][:].rearrange("p (k h q) -> p h k q", k=8, h=2)
                    pts = [None] * 4
                    ps = None
                    for kt in range(9):
                        if kt < 8:
                            # kt-PAIRED scores: one [128,1024] tile per kt
                            # pair; A halves (tile_position (0,0)) in bank X
                            # (cols 0:512), B halves ((64,0)) in bank Y (cols
                            # 512:1024). Only the even kt starts the group --
                            # its lazy zero region covers the odd kt's cols.
                            par = kt % 2
                            if par == 0:
                                ps = psc.tile([128, 1024], f32, tag="sc",
                                              name=f"sc{widx}_{kt // 2}")
                            nc.tensor.matmul(
                                ps[:, par * 256:(par + 1) * 256],
                                ktb[j][0:64, kt * 128:(kt + 1) * 128],
                                qtb[j][0:64, qs],
                                start=(par == 0), stop=(par == 1),
                                tile_position=(0, 0), skip_group_check=True)
                            nc.tensor.matmul(
                                ps[:, 512 + par * 256:512 + (par + 1) * 256],
                                ktb[j][64:128, kt * 128:(kt + 1) * 128],
                                qtb[j][64:128, qs],
                                start=(par == 0), stop=(par == 1),
                                tile_position=(64, 0), skip_group_check=True)
                        pend_avs(kt)
                        for u in fillers[kt]:
                            u()
                        if kt < 8 and kt % 2 == 1:
                            # one exp + one mask multiply per kt PAIR
                            pt = ppt.tile([128, 1024], bf16, tag="pt",
                                          name=f"pt{widx}_{kt // 2}")
                            nc.scalar.activation(pt[:], ps[:], AF.Exp)
                            nc.vector.tensor_mul(
                                pt[:].rearrange("p (h k q) -> p h k q",
                                                h=2, k=2),
                                pt[:].rearrange("p (h k q) -> p h k q",
                                                h=2, k=2),
                                mpv[:, :, kt - 1:kt + 1, qs])
                            pts[kt // 2] = pt
                    pend.update(pts=pts, j=j, qc=qc, widx=widx)

                # ---- out projection chunk (qt, ch): PSUM -> f16 -> DRAM ----
                def outproj(qt, ch):
                    ops = pps.tile([128, 512], f32, tag="pp",
                                   name=f"ops{qt}_{ch}")
                    for rcx in range(4):
                        nc.tensor.matmul(ops[:],
                                         attl[rcx][:, qt * 128:(qt + 1) * 128],
                                         wo3[:, rcx, ch * 512:(ch + 1) * 512],
                                         start=(rcx == 0), stop=(rcx == 3))
                    ot = pout.tile([128, 512], f16, tag="out",
                                   name=f"ot{qt}_{ch}")
                    if ch == 0:
                        nc.vector.tensor_copy(ot[:], ops[:])
                    else:
                        nc.scalar.activation(ot[:], ops[:], AF.Copy)
                    nc.sync.dma_start(
                        out=out[qt * 128:(qt + 1) * 128, ch * 512:(ch + 1) * 512],
                        in_=ot[:])

                # ---- schedule ----
                # head chains spread over the (still idle) psc/pav banks so
                # the 2-deep pps ring doesn't serialize the prologue
                qk_alloc(0)
                qk_piece(0, 0)
                qk_piece(0, 1, psc, "sc", [128, 1024])
                qk_piece(0, 2)
                qk_piece(0, 3, pav, "av")
                vchunk(0)
                vchunk(1)

                _vps = {}

                def vA(st):
                    def g():
                        _vps[st] = pps.tile([128, 512], f32, tag="pp",
                                            name=f"vps{st}")
                        for c8 in range(4):
                            nc.tensor.matmul(
                                _vps[st][:], xt3[:, c8, st * 128:(st + 1) * 128],
                                wv3[:, c8, :], start=(c8 == 0), stop=False)
                    return g

                def vB(st):
                    def g():
                        vps = _vps[st]
                        for c8 in range(4, 8):
                            nc.tensor.matmul(
                                vps[:], xt3[:, c8, st * 128:(st + 1) * 128],
                                wv3[:, c8, :], start=False, stop=(c8 == 7))
                        vt = pv.tile([128, 520], bf16, tag="vext",
                                     name=f"vext{st}")
                        v3 = vt[:].rearrange("p (h e) -> p h e", e=65)
                        if st % 2 == 0:
                            nc.scalar.activation(
                                v3[:, :, 0:64],
                                vps[:].rearrange("p (h e) -> p h e", e=64),
                                AF.Copy)
                        else:
                            nc.vector.tensor_copy(
                                v3[:, :, 0:64],
                                vps[:].rearrange("p (h e) -> p h e", e=64))
                        nc.gpsimd.memset(v3[:, :, 64:65], 1.0)
                        vext[st] = vt
                    return g

                def qkh(j, piece, half):
                    def g():
                        if piece == 0 and half == 0:
                            qk_alloc(j)
                        qk_half(j, piece, half)
                    return g

                def op(qt, ch):
                    return lambda: outproj(qt, ch)

                T = flush_transposes

                def slots(*units_at):
                    # units_at: dict slot -> list of units
                    f = [[] for _ in range(9)]
                    for s, us in units_at:
                        f[s].extend(us)
                    return f

                # per-window filler plans. Constraints: vext[kt] ready before
                # window 1 slot kt (all vchunks inside window 0); Q/K sh0 of
                # pair j evicted before window (j,0), K sh1 before its slot 4,
                # Q sh1 before (j,2); out-proj chunks once their attl quarter
                # (evicted 2 windows after the producing window) exists.
                F = [
                    slots((0, [vA(2)]), (1, [vB(2)]), (2, [vA(3)]),
                          (3, [vB(3)]), (4, [vA(4)]), (5, [vB(4)]),
                          (6, [vA(5)]), (7, [vB(5)])),
                    slots((0, [T]), (1, [vA(6)]), (2, [vB(6)]),
                          (3, [vA(7)]), (4, [vB(7)]),
                          (5, [qkh(1, 0, 0)]), (7, [qkh(1, 0, 1)])),
                    slots((0, [T]), (1, [qkh(1, 1, 0)]), (4, [qkh(1, 1, 1)])),
                    slots((0, [T]), (1, [qkh(1, 3, 0)]), (4, [qkh(1, 3, 1)])),
                    slots((0, [T]), (1, [qkh(1, 2, 0)]), (4, [qkh(1, 2, 1)])),
                    slots((0, [T]), (1, [qkh(2, 0, 0)]), (4, [qkh(2, 0, 1)])),
                    slots((0, [T]), (1, [qkh(2, 1, 0)]), (4, [qkh(2, 1, 1)])),
                    slots((0, [T]), (1, [qkh(2, 3, 0)]), (4, [qkh(2, 3, 1)])),
                    slots((0, [T]), (1, [qkh(2, 2, 0)]), (4, [qkh(2, 2, 1)])),
                    slots((0, [T]), (1, [qkh(3, 0, 0)]), (4, [qkh(3, 0, 1)])),
                    slots((0, [T]), (1, [qkh(3, 1, 0)]), (4, [qkh(3, 1, 1)])),
                    slots((0, [T]), (1, [qkh(3, 3, 0)]), (4, [qkh(3, 3, 1)])),
                    slots((0, [T]), (1, [qkh(3, 2, 0)]), (4, [qkh(3, 2, 1)])),
                    slots((0, [T])),
                    slots((0, [T]), (1, [op(0, 0)]), (3, [op(0, 1)]),
                          (5, [op(1, 0)]), (7, [op(1, 1)])),
                    slots((0, [T]), (1, [op(2, 0)]), (3, [op(2, 1)]),
                          (5, [op(3, 0)]), (7, [op(3, 1)])),
                ]

                widx = 0
                for j in range(4):
                    for qc in range(4):
                        window(j, qc, widx, F[widx])
                        widx += 1
                        if (j, qc) == (0, 1):
                            load_mask(2)
                        elif (j, qc) == (1, 1):
                            load_mask(3)

                # tail: flush window 14's transposes (enables qt 4/5 chunks),
                # run the final window's AVs interleaved with those chunks,
                # then its norm + transposes and the last out-proj chunks.
                flush_transposes()   # att of window 14 -> attl[3] cols 512:768
                tail_ops = [op(4, 0), op(4, 1), op(5, 0), op(5, 1)]
                for kt in range(9):
                    pend_avs(kt)
                    if kt in (1, 3, 5, 7):
                        tail_ops.pop(0)()
                # open the last four chunks (rcx 0-2 need only attl[0..2])
                # before the final transpose flush; close after it
                t_ops = []
                for qt, ch in ((6, 0), (6, 1), (7, 0), (7, 1)):
                    ops = pps.tile([128, 512], f32, tag="pp",
                                   name=f"ops{qt}_{ch}")
                    for rcx in range(3):
                        nc.tensor.matmul(
                            ops[:], attl[rcx][:, qt * 128:(qt + 1) * 128],
                            wo3[:, rcx, ch * 512:(ch + 1) * 512],
                            start=(rcx == 0), stop=False)
                    t_ops.append((qt, ch, ops))
                    if (qt, ch) == (6, 1):
                        flush_transposes()   # att of w15 -> attl[3] cols 768:
                for qt, ch, ops in t_ops:
                    nc.tensor.matmul(ops[:],
                                     attl[3][:, qt * 128:(qt + 1) * 128],
                                     wo3[:, 3, ch * 512:(ch + 1) * 512],
                                     start=False, stop=True)
                    ot = pout.tile([128, 512], f16, tag="out",
                                   name=f"otf{qt}_{ch}")
                    if ch == 0:
                        nc.vector.tensor_copy(ot[:], ops[:])
                    else:
                        nc.scalar.activation(ot[:], ops[:], AF.Copy)
                    nc.sync.dma_start(
                        out=out[qt * 128:(qt + 1) * 128,
                                ch * 512:(ch + 1) * 512],
                        in_=ot[:])

    nc.compile()
    return nc


def _get_nc():
    global _NC_CACHE
    if _NC_CACHE is None:
        _NC_CACHE = _build()
    return _NC_CACHE


def _prep_inputs(x, Wq, bq, Wk, bk, Wv, bv, Wo, bo, temperature, sparse_mask):
    bfd = ml_dtypes.bfloat16
    x = np.asarray(x, np.float32)
    Wq = np.asarray(Wq, np.float32); Wk = np.asarray(Wk, np.float32)
    Wv = np.asarray(Wv, np.float32); Wo = np.asarray(Wo, np.float32)
    bq = np.asarray(bq, np.float32); bk = np.asarray(bk, np.float32)
    temp = np.asarray(temperature, np.float32).reshape(-1)
    mask = np.asarray(sparse_mask)

    in_maps = []
    for c in CORE_IDS:
        b, g = c // 2, c % 2
        cols = slice(g * LOC, (g + 1) * LOC)
        hs = slice(g * GH, (g + 1) * GH)
        xt_h = np.ascontiguousarray(
            x[b].T.reshape(8, 128, 1024).transpose(1, 0, 2)).astype(bfd)
        wq_h = np.ascontiguousarray(
            Wq[:, cols].reshape(8, 128, 4, 128).transpose(1, 2, 0, 3)).astype(bfd)
        wk_h = np.ascontiguousarray(
            Wk[:, cols].reshape(8, 128, 4, 128).transpose(1, 2, 0, 3)).astype(bfd)
        wv_h = np.ascontiguousarray(
            Wv[:, cols].reshape(8, 128, 512).transpose(1, 0, 2)).astype(bfd)
        wo_h = np.ascontiguousarray(
            Wo[cols, :].reshape(4, 128, 1024).transpose(1, 0, 2)).astype(bfd)
        # mask pairs: [4 j, 128 p, 8 kt, 2 h, 1024 q]; element (j,p,kt,h,q) =
        # sparse_mask[b, hs[2j+h], q, kt*128+p]
        mt = mask[b, hs].transpose(0, 2, 1)            # [8h, 1024k, 1024q]
        mp_h = np.ascontiguousarray(
            mt.reshape(4, 2, 8, 128, 1024).transpose(0, 3, 2, 1, 4)
        ).astype(bfd).reshape(4, 128, 16384)
        cst_h = np.stack([
            (np.repeat(temp[hs], D) / np.sqrt(D)).astype(np.float32),
            bq[cols].astype(np.float32),
            bk[cols].astype(np.float32),
        ]).reshape(3, 4, 128).transpose(2, 0, 1).reshape(128, 12)
        in_maps.append({
            "xt": xt_h, "wq": wq_h, "wk": wk_h, "wv": wv_h, "wo": wo_h,
            "mp": mp_h, "cst": np.ascontiguousarray(cst_h),
        })
    return in_maps


def kernel(**inputs):
    in_maps = _prep_inputs(**inputs)
    nc = _get_nc()
    res = run_bass_kernel_spmd(nc, in_maps, CORE_IDS)
    # unshard: row-parallel partial sum per batch + constant bias row
    # (softmax rows sum to 1 so bv contributes bv @ Wo to every row)
    bv = np.asarray(inputs["bv"], np.float32)
    bo = np.asarray(inputs["bo"], np.float32)
    Wo = np.asarray(inputs["Wo"], np.float32)
    brow = bv @ Wo + bo
    out = np.empty((B, S, HID), np.float32)
    for b in range(B):
        out[b] = (res.results[2 * b]["out"].astype(np.float32)
                  + res.results[2 * b + 1]["out"].astype(np.float32) + brow)
    return out


# revision 66
# speedup vs baseline: 1.1014x; 1.0096x over previous
"""Trainium2 Bass kernel for nn_AdaptiveAttention (sparse attention, B=4 S=1024 HID=1024 H=16).

Sharding (8 cores): core c = (batch b=c//2) x (head-group g=c%2, 8 heads / 512 hid cols).

v2 design (cost-model driven; ~119.8us vs 151.9us v1 baseline):
- All DRAM inputs host-pre-tiled into exact SBUF layouts so every DMA is a
  contiguous >=1KB-run burst (full-rate in the DMA model; elem runs >=512B).
- Q^T/K^T = W x x^T with temperature/sqrt(D) folded into the Q eviction
  (DVE tensor_scalar).
- Attention runs in 16 quarter-windows (j head-pair x qc 256-q columns):
  scores are kt-PAIRED into one [128,1024] PSUM tile per two k-tiles: the A
  (tile_position (0,0)) halves fill bank X (cols 0:512), the B ((64,0))
  halves bank Y (cols 512:1024) -- a tile_position pair sharing a bank, or
  any start at a non-bank-aligned offset, crashes the hw, but one group per
  bank (start only on the even kt, lazy zero-region covering the odd kt's
  cols) is legal. One exp (ACT) and one mask-multiply (DVE 2x bf16) then
  cover 2 heads x 2 k-tiles, halving ACT op count (64 exps total).
- AV restructured: stationary = P^T tile [128k,128q], moving = Vext [128k,65]
  (ones column) -> av [128q,65] accumulated in one bank-sized PSUM tile as a
  SINGLE accumulation group (start only on the first matmul: the lazy
  zero-region covers all 4 chains; stop only on the last). Halves AV
  tensor-engine rows (ap=65 vs 512; LdWeights is free in the cost model) and
  makes the softmax denominator a per-PARTITION column: normalization is one
  reciprocal + one broadcast multiply -- no DMA broadcasts at all.
- Each window's AV matmuls run one FULL window later (pt tiles are a window
  old, so the exp->mask chain can never stall them); att[q,d] returns to
  attT[d,q] via PE transposes (4/window) batched in a dedicated PSUM bank
  with one [128,256] eviction into attl.
- Junk warm-up matmuls ramp the PE p-state during the DMA prologue; V-chunk
  evictions alternate ACT/DVE; out-projection chunks evict to f16 (halves
  output DMA) alternating ACT/DVE, host sums partials + (bv@Wo+bo) row.
- DMA queue discipline matters as much as bytes: the gpsimd/SWDGE queue
  carries ONLY the mask streams (its per-DMA ~1us Pool desc-gen would
  otherwise delay mask kt-tiles and head-of-line-block window 0's mask
  multiplies -> DVE FIFO -> eviction ring -> PE); the 3 tiny consts ride the
  sync/HWDGE queue ahead of weights as ONE packed [128,12] DMA (three
  separate small DMAs cost 3x the ~630ns HWDGE issue serialization), and wv
  precedes the j1-3 W slices so the V chunks are never input-gated.
- PSUM = exactly 8 banks: scores 2x[128,1024] + av [128,512] + transpose
  batch [128,1024]bf16 + 2x[128,512] general ring (projections/V/out-proj).
"""
import os
import sys

for _p in ("/opt/trn_rl_repo", "/root/.axon_site/_ro/trn_rl_repo"):
    if os.path.isdir(_p) and _p not in sys.path:
        sys.path.insert(0, _p)

import numpy as np
import ml_dtypes

import concourse.bass as bass
from concourse import bacc
import concourse.mybir as mybir
import concourse.tile as tile
from concourse.bass_utils import run_bass_kernel_spmd
from concourse.masks import make_identity

B, S, HID, H, D = 4, 1024, 1024, 16, 64
NCORES = 8
GH = 8          # heads per core
LOC = GH * D    # 512, local hid slice
CORE_IDS = list(range(NCORES))

bf16 = mybir.dt.bfloat16
f32 = mybir.dt.float32
AF = mybir.ActivationFunctionType
ALU = mybir.AluOpType

_NC_CACHE = None


def _build(dbg=False, reps=1):
    nc = bacc.Bacc("TRN2", debug=False, num_devices=NCORES)

    xt = nc.declare_dram_parameter("xt", [128, 8, 1024], bf16, False)
    wq = nc.declare_dram_parameter("wq", [128, 4, 8, 128], bf16, False)
    wk = nc.declare_dram_parameter("wk", [128, 4, 8, 128], bf16, False)
    wv = nc.declare_dram_parameter("wv", [128, 8, 512], bf16, False)
    wo = nc.declare_dram_parameter("wo", [128, 4, 1024], bf16, False)
    mp = nc.declare_dram_parameter("mp", [4, 128, 8 * 2 * 1024], bf16, False)
    cst = nc.declare_dram_parameter("cst", [128, 12], f32, False)
    f16 = mybir.dt.float16
    out = nc.declare_dram_parameter("out", [S, HID], f16, True)    # partial

    with tile.TileContext(nc) as tc:
        with (
            tc.tile_pool(name="pw", bufs=4) as pw,           # weights
            tc.tile_pool(name="pxt", bufs=1) as pxt,         # xT [128,8192] bf16
            tc.tile_pool(name="pqk", bufs=8) as pqk,         # QT/KT [128,1024] bf16
            tc.tile_pool(name="pv", bufs=8) as pv,           # Vext [128,520] bf16
            tc.tile_pool(name="ppt", bufs=10) as ppt,        # P^T kt-pair [128,1024] bf16
            tc.tile_pool(name="pmask", bufs=2) as pmask,     # mask pair [128,16384] bf16
            tc.tile_pool(name="patt", bufs=4) as patt,       # att [128,256] bf16
            tc.tile_pool(name="pattl", bufs=4) as pattl,     # attT per j (live to end)
            tc.tile_pool(name="prcp", bufs=3) as prcp,       # recip [128,4] f32
            tc.tile_pool(name="pout", bufs=4) as pout,       # out staging f16
            tc.tile_pool(name="pconst", bufs=1) as pconst,   # small tiles
            tc.tile_pool(name="psc", bufs=2, space="PSUM") as psc,   # scores [128,1024]
            tc.tile_pool(name="pav", bufs=1, space="PSUM") as pav,   # av [128,512]
            tc.tile_pool(name="ptrp", bufs=1, space="PSUM") as ptrp, # transposes
            tc.tile_pool(name="pps", bufs=2, space="PSUM") as pps,   # general ring
        ):
            for _rep in range(reps):
                # ---- small constants (gpsimd queue, ahead of masks) ----
                # warm-up memset first: no deps, so the PE warm-up isn't
                # queued behind the const DMAs on DVE
                wrm = pconst.tile([128, 128], bf16, tag="wrm")
                nc.vector.memset(wrm[:], 0.0)

                # identity for PE transposes
                ident = pconst.tile([128, 128], bf16, tag="ident")
                make_identity(nc, ident[:])

                # ---- PE warm-up: junk matmuls ramp the p-state while DMAs
                # stream; their PSUM tile is write-only and recycled ----
                wps = pps.tile([128, 512], f32, tag="pp", name="warmps")
                for _w in range(48):
                    nc.tensor.matmul(wps[:, 0:128], wrm[:], wrm[:],
                                     start=True, stop=True)

                # ---- bulk loads (sync queue = HWDGE) ----
                wqb = pw.tile([128, 4 * 8 * 128], bf16, tag="w", name="wq")
                wq4 = wqb[:].rearrange("p (j c n) -> p j c n", j=4, c=8)
                wkb = pw.tile([128, 4 * 8 * 128], bf16, tag="w", name="wk")
                wk4 = wkb[:].rearrange("p (j c n) -> p j c n", j=4, c=8)
                xtb = pxt.tile([128, 8 * 1024], bf16, tag="xt", name="xt")
                xt3 = xtb[:].rearrange("p (c s) -> p c s", c=8)

                nc.sync.dma_start(out=wq4[:, 0], in_=wq[:, 0])
                nc.sync.dma_start(out=xt3[:, 0:4, 0:512], in_=xt[:, 0:4, 0:512])
                cst_t = pconst.tile([128, 12], f32, tag="cst", name="cst")
                nc.sync.dma_start(out=cst_t[:], in_=cst[:])
                scale_t = cst_t[:, 0:4]
                bq_t = cst_t[:, 4:8]
                bk_t = cst_t[:, 8:12]
                bqs_t = pconst.tile([128, 4], f32, tag="bqs")
                nc.vector.tensor_mul(bqs_t[:], bq_t, scale_t)
                # preload exp table during the DMA phase
                warm_t = pconst.tile([1, 4], f32, tag="warmexp")
                nc.scalar.activation(warm_t[:], scale_t[0:1, :], AF.Exp)
                nc.sync.dma_start(out=xt3[:, 4:8, 0:512], in_=xt[:, 4:8, 0:512])
                nc.sync.dma_start(out=wk4[:, 0], in_=wk[:, 0])
                nc.sync.dma_start(out=xt3[:, :, 512:1024],
                                  in_=xt[:, :, 512:1024])
                wvb = pw.tile([128, 8 * 512], bf16, tag="w", name="wv")
                wv3 = wvb[:].rearrange("p (c n) -> p c n", c=8)
                nc.sync.dma_start(out=wv3[:], in_=wv[:])
                for _j in (1, 2, 3):
                    nc.sync.dma_start(out=wq4[:, _j], in_=wq[:, _j])
                for _j in (1, 2, 3):
                    nc.sync.dma_start(out=wk4[:, _j], in_=wk[:, _j])
                wob = pw.tile([128, 4 * 1024], bf16, tag="w", name="wo")
                wo3 = wob[:].rearrange("p (r n) -> p r n", r=4)
                nc.sync.dma_start(out=wo3[:], in_=wo[:])

                # ---- mask pair loads (gpsimd queue), kt-granular ----
                mh = [None] * 4

                def load_mask(j):
                    t = pmask.tile([128, 8 * 2 * 1024], bf16, tag="mask",
                                   name=f"mp{j}")
                    tv = t[:].rearrange("p (k x) -> p k x", k=8)
                    for kt in range(8):
                        nc.gpsimd.dma_start(
                            out=tv[:, kt],
                            in_=mp[j, :, kt * 2048:(kt + 1) * 2048])
                    mh[j] = t

                load_mask(0)
                load_mask(1)

                # ---- Q^T/K^T projection pieces ----
                qtb = [None] * 4
                ktb = [None] * 4

                def qk_alloc(j):
                    qtb[j] = pqk.tile([128, 1024], bf16, tag="qk", name=f"qt{j}")
                    ktb[j] = pqk.tile([128, 1024], bf16, tag="qk", name=f"kt{j}")

                _qk_ps = {}

                # piece 0: Q s-half0, 1: K s-half0, 2: Q s-half1, 3: K s-half1
                def qk_half(j, piece, half, pool=None, ptag="pp", pshape=None):
                    is_q = piece % 2 == 0
                    sh = piece // 2
                    wsrc = wq4 if is_q else wk4
                    key = (j, piece)
                    if half == 0:
                        _qk_ps[key] = (pool or pps).tile(
                            pshape or [128, 512], f32, tag=ptag,
                            name=f"qkps{j}_{piece}")
                    psa = _qk_ps[key][:, 0:512]
                    for c8 in range(4 * half, 4 * half + 4):
                        nc.tensor.matmul(psa, wsrc[:, j, c8, :],
                                         xt3[:, c8, sh * 512:(sh + 1) * 512],
                                         start=(c8 == 0), stop=(c8 == 7))
                    if half == 1:
                        dst = (qtb[j] if is_q else ktb[j])[:, sh * 512:(sh + 1) * 512]
                        if is_q:
                            nc.vector.tensor_scalar(
                                dst, psa, cst_t[:, j:j + 1],
                                bqs_t[:, j:j + 1], ALU.mult, ALU.add)
                        else:
                            nc.vector.tensor_scalar_add(dst, psa,
                                                        cst_t[:, 8 + j:9 + j])

                def qk_piece(j, piece, pool=None, ptag="pp", pshape=None):
                    qk_half(j, piece, 0, pool, ptag, pshape)
                    qk_half(j, piece, 1, pool, ptag, pshape)

                # ---- V projection chunk st -> Vext [128 s, 8*65] ----
                vext = [None] * 8

                def vchunk(st, pool=None, ptag="pp", pshape=None):
                    vps = (pool or pps).tile(pshape or [128, 512], f32,
                                             tag=ptag, name=f"vps{st}")
                    vps = vps[:, 0:512]
                    for c8 in range(8):
                        nc.tensor.matmul(vps, xt3[:, c8, st * 128:(st + 1) * 128],
                                         wv3[:, c8, :], start=(c8 == 0), stop=(c8 == 7))
                    vt = pv.tile([128, 520], bf16, tag="vext", name=f"vext{st}")
                    v3 = vt[:].rearrange("p (h e) -> p h e", e=65)
                    # alternate eviction engine so neither ACT nor DVE piles up
                    if st % 2 == 0:
                        nc.scalar.activation(
                            v3[:, :, 0:64],
                            vps.rearrange("p (h e) -> p h e", e=64), AF.Copy)
                    else:
                        nc.vector.tensor_copy(
                            v3[:, :, 0:64],
                            vps.rearrange("p (h e) -> p h e", e=64))
                    nc.gpsimd.memset(v3[:, :, 64:65], 1.0)
                    vext[st] = vt

                # ---- attention machinery ----
                attl = [None] * 4
                pending_tr = []   # deferred (j, qc, att_tile, widx)

                def flush_transposes():
                    if not pending_tr:
                        return
                    j, qc, att, w = pending_tr.pop(0)
                    ptr_t = ptrp.tile([128, 1024], bf16, tag="ptr",
                                     name=f"ptr{w}")
                    for h in (0, 1):
                        for qt in (0, 1):
                            nc.tensor.transpose(
                                ptr_t[h * 64:(h + 1) * 64,
                                      qt * 128:(qt + 1) * 128],
                                att[:, (h * 2 + qt) * 64:(h * 2 + qt + 1) * 64],
                                ident[:])
                    if attl[j] is None:
                        attl[j] = pattl.tile([128, 1024], bf16, tag="attl",
                                             name=f"attl{j}")
                    nc.vector.tensor_copy(attl[j][:, qc * 256:(qc + 1) * 256],
                                          ptr_t[:, 0:256])

                # pending AV state: the avs of window w run one window later,
                # by which time every pt tile is ~a full window old, so they
                # never wait on the exp/mask chain.
                pend = {}   # pts, j, qc, widx, avt

                def pend_avs(kt):
                    # one kt's worth (4 matmuls) of the previous window's AV
                    if not pend:
                        return
                    if kt == 8:
                        # normalization: per-partition reciprocal+scalar mult
                        avt, pj, pqc, pw = (pend["avt"], pend["j"],
                                            pend["qc"], pend["widx"])
                        att = patt.tile([128, 256], bf16, tag="att",
                                        name=f"att{pw}")
                        rcp = prcp.tile([128, 4], f32, tag="rcp",
                                        name=f"rcp{pw}")
                        avv = avt[:, 0:260].rearrange("p (n e) -> p n e", e=65)
                        nc.vector.reciprocal(rcp[:], avv[:, :, 64])
                        # one broadcast multiply normalizes all 4 chains
                        nc.vector.tensor_mul(
                            att[:].rearrange("p (n e) -> p n e", e=64),
                            avv[:, :, 0:64],
                            rcp[:].to_broadcast([128, 4, 64]))
                        pending_tr.append((pj, pqc, att, pw))
                        pend.clear()
                        return
                    if kt == 0:
                        # single bank-sized av tile, one accumulation group:
                        # the first matmul's start lazily zeroes the whole 2KB
                        # zero region so all 4 chains accumulate from zero.
                        pend["avt"] = pav.tile([128, 512], f32, tag="av",
                                               name=f"av{pend['widx']}")
                    avt = pend["avt"]
                    ptk = pend["pts"][kt // 2]
                    par = kt % 2
                    pj = pend["j"]
                    for h in (0, 1):
                        hh = 2 * pj + h
                        for qt in (0, 1):
                            c = (h * 2 + qt) * 65
                            nc.tensor.matmul(
                                avt[:, c:c + 65],
                                ptk[:, h * 512 + par * 256 + qt * 128:
                                    h * 512 + par * 256 + (qt + 1) * 128],
                                vext[kt][:, hh * 65:(hh + 1) * 65],
                                start=(kt == 0 and h == 0 and qt == 0),
                                stop=(kt == 7 and h == 1 and qt == 1),
                                skip_group_check=True)

                def window(j, qc, widx, fillers):
                    qs = slice(qc * 256, (qc + 1) * 256)
                    # mask view permuted to (h, kt, q) to match pt layout
                    mpv = mh[j][:].rearrange("p (k h q) -> p h k q", k=8, h=2)
                    pts = [None] * 4
                    ps = None
                    for kt in range(9):
                        if kt < 8:
                            # kt-PAIRED scores: one [128,1024] tile per kt
                            # pair; A halves (tile_position (0,0)) in bank X
                            # (cols 0:512), B halves ((64,0)) in bank Y (cols
                            # 512:1024). Only the even kt starts the group --
                            # its lazy zero region covers the odd kt's cols.
                            par = kt % 2
                            if par == 0:
                                ps = psc.tile([128, 1024], f32, tag="sc",
                                              name=f"sc{widx}_{kt // 2}")
                            nc.tensor.matmul(
                                ps[:, par * 256:(par + 1) * 256],
                                ktb[j][0:64, kt * 128:(kt + 1) * 128],
                                qtb[j][0:64, qs],
                                start=(par == 0), stop=(par == 1),
                                tile_position=(0, 0), skip_group_check=True)
                            nc.tensor.matmul(
                                ps[:, 512 + par * 256:512 + (par + 1) * 256],
                                ktb[j][64:128, kt * 128:(kt + 1) * 128],
                                qtb[j][64:128, qs],
                                start=(par == 0), stop=(par == 1),
                                tile_position=(64, 0), skip_group_check=True)
                        pend_avs(kt)
                        for u in fillers[kt]:
                            u()
                        if kt < 8 and kt % 2 == 1:
                            # one exp + one mask multiply per kt PAIR
                            pt = ppt.tile([128, 1024], bf16, tag="pt",
                                          name=f"pt{widx}_{kt // 2}")
                            nc.scalar.activation(pt[:], ps[:], AF.Exp)
                            nc.vector.tensor_mul(
                                pt[:].rearrange("p (h k q) -> p h k q",
                                                h=2, k=2),
                                pt[:].rearrange("p (h k q) -> p h k q",
                                                h=2, k=2),
                                mpv[:, :, kt - 1:kt + 1, qs])
                            pts[kt // 2] = pt
                    pend.update(pts=pts, j=j, qc=qc, widx=widx)

                # ---- out projection chunk (qt, ch): PSUM -> f16 -> DRAM ----
                def outproj(qt, ch):
                    ops = pps.tile([128, 512], f32, tag="pp",
                                   name=f"ops{qt}_{ch}")
                    for rcx in range(4):
                        nc.tensor.matmul(ops[:],
                                         attl[rcx][:, qt * 128:(qt + 1) * 128],
                                         wo3[:, rcx, ch * 512:(ch + 1) * 512],
                                         start=(rcx == 0), stop=(rcx == 3))
                    ot = pout.tile([128, 512], f16, tag="out",
                                   name=f"ot{qt}_{ch}")
                    if ch == 0:
                        nc.vector.tensor_copy(ot[:], ops[:])
                    else:
                        nc.scalar.activation(ot[:], ops[:], AF.Copy)
                    nc.sync.dma_start(
                        out=out[qt * 128:(qt + 1) * 128, ch * 512:(ch + 1) * 512],
                        in_=ot[:])

                # ---- schedule ----
                # head chains spread over the (still idle) psc/pav banks so
                # the 2-deep pps ring doesn't serialize the prologue
                qk_alloc(0)
                qk_piece(0, 0)
                qk_piece(0, 1, psc, "sc", [128, 1024])
                qk_piece(0, 2)
                qk_piece(0, 3, pav, "av")
                vchunk(0)
                vchunk(1)

                _vps = {}

                def vA(st):
                    def g():
                        _vps[st] = pps.tile([128, 512], f32, tag="pp",
                                            name=f"vps{st}")
                        for c8 in range(4):
                            nc.tensor.matmul(
                                _vps[st][:], xt3[:, c8, st * 128:(st + 1) * 128],
                                wv3[:, c8, :], start=(c8 == 0), stop=False)
                    return g

                def vB(st):
                    def g():
                        vps = _vps[st]
                        for c8 in range(4, 8):
                            nc.tensor.matmul(
                                vps[:], xt3[:, c8, st * 128:(st + 1) * 128],
                                wv3[:, c8, :], start=False, stop=(c8 == 7))
                        vt = pv.tile([128, 520], bf16, tag="vext",
                                     name=f"vext{st}")
                        v3 = vt[:].rearrange("p (h e) -> p h e", e=65)
                        if st % 2 == 0:
                            nc.scalar.activation(
                                v3[:, :, 0:64],
                                vps[:].rearrange("p (h e) -> p h e", e=64),
                                AF.Copy)
                        else:
                            nc.vector.tensor_copy(
                                v3[:, :, 0:64],
                                vps[:].rearrange("p (h e) -> p h e", e=64))
                        nc.gpsimd.memset(v3[:, :, 64:65], 1.0)
                        vext[st] = vt
                    return g

                def qkh(j, piece, half):
                    def g():
                        if piece == 0 and half == 0:
                            qk_alloc(j)
                        qk_half(j, piece, half)
                    return g

                def op(qt, ch):
                    return lambda: outproj(qt, ch)

                T = flush_transposes

                def slots(*units_at):
                    # units_at: dict slot -> list of units
                    f = [[] for _ in range(9)]
                    for s, us in units_at:
                        f[s].extend(us)
                    return f

                # per-window filler plans. Constraints: vext[kt] ready before
                # window 1 slot kt (all vchunks inside window 0); Q/K sh0 of
                # pair j evicted before window (j,0), K sh1 before its slot 4,
                # Q sh1 before (j,2); out-proj chunks once their attl quarter
                # (evicted 2 windows after the producing window) exists.
                F = [
                    slots((0, [vA(2)]), (1, [vB(2)]), (2, [vA(3)]),
                          (3, [vB(3)]), (4, [vA(4)]), (5, [vB(4)]),
                          (6, [vA(5)]), (7, [vB(5)])),
                    slots((0, [T]), (1, [vA(6)]), (2, [vB(6)]),
                          (3, [vA(7)]), (4, [vB(7)]),
                          (5, [qkh(1, 0, 0)]), (7, [qkh(1, 0, 1)])),
                    slots((0, [T]), (1, [qkh(1, 1, 0)]), (4, [qkh(1, 1, 1)])),
                    slots((0, [T]), (1, [qkh(1, 3, 0)]), (4, [qkh(1, 3, 1)])),
                    slots((0, [T]), (1, [qkh(1, 2, 0)]), (4, [qkh(1, 2, 1)])),
                    slots((0, [T]), (1, [qkh(2, 0, 0)]), (4, [qkh(2, 0, 1)])),
                    slots((0, [T]), (1, [qkh(2, 1, 0)]), (4, [qkh(2, 1, 1)])),
                    slots((0, [T]), (1, [qkh(2, 3, 0)]), (4, [qkh(2, 3, 1)])),
                    slots((0, [T]), (1, [qkh(2, 2, 0)]), (4, [qkh(2, 2, 1)])),
                    slots((0, [T]), (1, [qkh(3, 0, 0)]), (4, [qkh(3, 0, 1)])),
                    slots((0, [T]), (1, [qkh(3, 1, 0)]), (4, [qkh(3, 1, 1)])),
                    slots((0, [T]), (1, [qkh(3, 3, 0)]), (4, [qkh(3, 3, 1)])),
                    slots((0, [T]), (1, [qkh(3, 2, 0)]), (4, [qkh(3, 2, 1)])),
                    slots((0, [T])),
                    slots((0, [T]), (1, [op(0, 0)]), (3, [op(0, 1)]),
                          (5, [op(1, 0)]), (7, [op(1, 1)])),
                    slots((0, [T]), (1, [op(2, 0)]), (3, [op(2, 1)]),
                          (5, [op(3, 0)]), (7, [op(3, 1)])),
                ]

                widx = 0
                for j in range(4):
                    for qc in range(4):
                        window(j, qc, widx, F[widx])
                        widx += 1
                        if (j, qc) == (0, 1):
                            load_mask(2)
                        elif (j, qc) == (1, 1):
                            load_mask(3)

                # tail: flush window 14's transposes (enables qt 4/5 chunks),
                # run the final window's AVs interleaved with those chunks,
                # then its norm + transposes and the last out-proj chunks.
                flush_transposes()   # att of window 14 -> attl[3] cols 512:768
                tail_ops = [op(4, 0), op(4, 1), op(5, 0), op(5, 1)]
                for kt in range(9):
                    pend_avs(kt)
                    if kt in (1, 3, 5, 7):
                        tail_ops.pop(0)()
                # open the last four chunks (rcx 0-2 need only attl[0..2])
                # before the final transpose flush; close after it
                t_ops = []
                for qt, ch in ((6, 0), (6, 1), (7, 0), (7, 1)):
                    ops = pps.tile([128, 512], f32, tag="pp",
                                   name=f"ops{qt}_{ch}")
                    for rcx in range(3):
                        nc.tensor.matmul(
                            ops[:], attl[rcx][:, qt * 128:(qt + 1) * 128],
                            wo3[:, rcx, ch * 512:(ch + 1) * 512],
                            start=(rcx == 0), stop=False)
                    t_ops.append((qt, ch, ops))
                    if (qt, ch) == (6, 1):
                        flush_transposes()   # att of w15 -> attl[3] cols 768:
                for qt, ch, ops in t_ops:
                    nc.tensor.matmul(ops[:],
                                     attl[3][:, qt * 128:(qt + 1) * 128],
                                     wo3[:, 3, ch * 512:(ch + 1) * 512],
                                     start=False, stop=True)
                    ot = pout.tile([128, 512], f16, tag="out",
                                   name=f"otf{qt}_{ch}")
                    if ch == 0:
                        nc.vector.tensor_copy(ot[:], ops[:])
                    else:
                        nc.scalar.activation(ot[:], ops[:], AF.Copy)
                    nc.sync.dma_start(
                        out=out[qt * 128:(qt + 1) * 128,
                                ch * 512:(ch + 1) * 512],
                        in_=ot[:])

    nc.compile()
    return nc


def _get_nc():
    global _NC_CACHE
    if _NC_CACHE is None:
        _NC_CACHE = _build()
    return _NC_CACHE


def _prep_inputs(x, Wq, bq, Wk, bk, Wv, bv, Wo, bo, temperature, sparse_mask):
    bfd = ml_dtypes.bfloat16
    x = np.asarray(x, np.float32)
    Wq = np.asarray(Wq, np.float32); Wk = np.asarray(Wk, np.float32)
    Wv = np.asarray(Wv, np.float32); Wo = np.asarray(Wo, np.float32)
    bq = np.asarray(bq, np.float32); bk = np.asarray(bk, np.float32)
    temp = np.asarray(temperature, np.float32).reshape(-1)
    mask = np.asarray(sparse_mask)

    in_maps = []
    for c in CORE_IDS:
        b, g = c // 2, c % 2
        cols = slice(g * LOC, (g + 1) * LOC)
        hs = slice(g * GH, (g + 1) * GH)
        xt_h = np.ascontiguousarray(
            x[b].T.reshape(8, 128, 1024).transpose(1, 0, 2)).astype(bfd)
        wq_h = np.ascontiguousarray(
            Wq[:, cols].reshape(8, 128, 4, 128).transpose(1, 2, 0, 3)).astype(bfd)
        wk_h = np.ascontiguousarray(
            Wk[:, cols].reshape(8, 128, 4, 128).transpose(1, 2, 0, 3)).astype(bfd)
        wv_h = np.ascontiguousarray(
            Wv[:, cols].reshape(8, 128, 512).transpose(1, 0, 2)).astype(bfd)
        wo_h = np.ascontiguousarray(
            Wo[cols, :].reshape(4, 128, 1024).transpose(1, 0, 2)).astype(bfd)
        # mask pairs: [4 j, 128 p, 8 kt, 2 h, 1024 q]; element (j,p,kt,h,q) =
        # sparse_mask[b, hs[2j+h], q, kt*128+p]
        mt = mask[b, hs].transpose(0, 2, 1)            # [8h, 1024k, 1024q]
        mp_h = np.ascontiguousarray(
            mt.reshape(4, 2, 8, 128, 1024).transpose(0, 3, 2, 1, 4)
        ).astype(bfd).reshape(4, 128, 16384)
        cst_h = np.stack([
            (np.repeat(temp[hs], D) / np.sqrt(D)).astype(np.float32),
            bq[cols].astype(np.float32),
            bk[cols].astype(np.float32),
        ]).reshape(3, 4, 128).transpose(2, 0, 1).reshape(128, 12)
        in_maps.append({
            "xt": xt_h, "wq": wq_h, "wk": wk_h, "wv": wv_h, "wo": wo_h,
            "mp": mp_h, "cst": np.ascontiguousarray(cst_h),
        })
    return in_maps


def kernel(**inputs):
    in_maps = _prep_inputs(**inputs)
    nc = _get_nc()
    res = run_bass_kernel_spmd(nc, in_maps, CORE_IDS)
    # unshard: row-parallel partial sum per batch + constant bias row
    # (softmax rows sum to 1 so bv contributes bv @ Wo to every row)
    bv = np.asarray(inputs["bv"], np.float32)
    bo = np.asarray(inputs["bo"], np.float32)
    Wo = np.asarray(inputs["Wo"], np.float32)
    brow = bv @ Wo + bo
    out = np.empty((B, S, HID), np.float32)
    for b in range(B):
        out[b] = (res.results[2 * b]["out"].astype(np.float32)
                  + res.results[2 * b + 1]["out"].astype(np.float32) + brow)
    return out


# revision 71
# speedup vs baseline: 1.1092x; 1.0071x over previous
"""Trainium2 Bass kernel for nn_AdaptiveAttention (sparse attention, B=4 S=1024 HID=1024 H=16).

Sharding (8 cores): core c = (batch b=c//2) x (head-group g=c%2, 8 heads / 512 hid cols).

v2 design (cost-model driven; ~118.7us vs 151.9us v1 baseline):
- All DRAM inputs host-pre-tiled into exact SBUF layouts so every DMA is a
  contiguous >=1KB-run burst (full-rate in the DMA model; elem runs >=512B).
- Q^T/K^T = W x x^T with temperature/sqrt(D) folded into the Q eviction
  (DVE tensor_scalar).
- Attention runs in 16 quarter-windows (j head-pair x qc 256-q columns):
  scores are kt-PAIRED into one [128,1024] PSUM tile per two k-tiles: the A
  (tile_position (0,0)) halves fill bank X (cols 0:512), the B ((64,0))
  halves bank Y (cols 512:1024) -- a tile_position pair sharing a bank, or
  any start at a non-bank-aligned offset, crashes the hw, but one group per
  bank (start only on the even kt, lazy zero-region covering the odd kt's
  cols) is legal. One exp (ACT) and one mask-multiply (DVE 2x bf16) then
  cover 2 heads x 2 k-tiles, halving ACT op count (64 exps total).
- AV restructured: stationary = P^T tile [128k,128q], moving = Vext [128k,65]
  (ones column) -> av [128q,65] accumulated in one bank-sized PSUM tile as a
  SINGLE accumulation group (start only on the first matmul: the lazy
  zero-region covers all 4 chains; stop only on the last). Halves AV
  tensor-engine rows (ap=65 vs 512; LdWeights is free in the cost model) and
  makes the softmax denominator a per-PARTITION column: normalization is one
  reciprocal + one broadcast multiply -- no DMA broadcasts at all.
- Each window's AV matmuls run one FULL window later (pt tiles are a window
  old, so the exp->mask chain can never stall them); att[q,d] returns to
  attT[d,q] via PE transposes (4/window) batched in a dedicated PSUM bank
  with one [128,256] eviction into attl.
- Junk warm-up matmuls ramp the PE p-state during the DMA prologue; V-chunk
  evictions alternate ACT/DVE; out-projection chunks evict to f16 (halves
  output DMA) alternating ACT/DVE, host sums partials + (bv@Wo+bo) row.
- DMA queue discipline matters as much as bytes: the gpsimd/SWDGE queue
  carries ONLY the mask streams (its per-DMA ~1us Pool desc-gen would
  otherwise delay mask kt-tiles and head-of-line-block window 0's mask
  multiplies -> DVE FIFO -> eviction ring -> PE); the 3 tiny consts ride the
  sync/HWDGE queue ahead of weights as ONE packed [128,12] DMA (three
  separate small DMAs cost 3x the ~630ns HWDGE issue serialization), and wv
  precedes the j1-3 W slices so the V chunks are never input-gated.
- PSUM = exactly 8 banks: scores 2x[128,1024] + av [128,512] + transpose
  batch [128,1024]bf16 + 2x[128,512] general ring (projections/V/out-proj).
"""
import os
import sys

for _p in ("/opt/trn_rl_repo", "/root/.axon_site/_ro/trn_rl_repo"):
    if os.path.isdir(_p) and _p not in sys.path:
        sys.path.insert(0, _p)

import numpy as np
import ml_dtypes

import concourse.bass as bass
from concourse import bacc
import concourse.mybir as mybir
import concourse.tile as tile
from concourse.bass_utils import run_bass_kernel_spmd
from concourse.masks import make_identity

B, S, HID, H, D = 4, 1024, 1024, 16, 64
NCORES = 8
GH = 8          # heads per core
LOC = GH * D    # 512, local hid slice
CORE_IDS = list(range(NCORES))

bf16 = mybir.dt.bfloat16
f32 = mybir.dt.float32
AF = mybir.ActivationFunctionType
ALU = mybir.AluOpType

_NC_CACHE = None


def _build(dbg=False, reps=1):
    nc = bacc.Bacc("TRN2", debug=False, num_devices=NCORES)

    xt = nc.declare_dram_parameter("xt", [128, 8, 1024], bf16, False)
    wq = nc.declare_dram_parameter("wq", [128, 4, 8, 128], bf16, False)
    wk = nc.declare_dram_parameter("wk", [128, 4, 8, 128], bf16, False)
    wv = nc.declare_dram_parameter("wv", [128, 8, 512], bf16, False)
    wo = nc.declare_dram_parameter("wo", [128, 4, 1024], bf16, False)
    mp = nc.declare_dram_parameter("mp", [4, 128, 8 * 2 * 1024], bf16, False)
    cst = nc.declare_dram_parameter("cst", [128, 12], f32, False)
    f16 = mybir.dt.float16
    out = nc.declare_dram_parameter("out", [S, HID], f16, True)    # partial

    with tile.TileContext(nc) as tc:
        with (
            tc.tile_pool(name="pw", bufs=4) as pw,           # weights
            tc.tile_pool(name="pxt", bufs=1) as pxt,         # xT [128,8192] bf16
            tc.tile_pool(name="pqk", bufs=8) as pqk,         # QT/KT [128,1024] bf16
            tc.tile_pool(name="pv", bufs=8) as pv,           # Vext [128,520] bf16
            tc.tile_pool(name="ppt", bufs=10) as ppt,        # P^T kt-pair [128,1024] bf16
            tc.tile_pool(name="pmask", bufs=2) as pmask,     # mask pair [128,16384] bf16
            tc.tile_pool(name="patt", bufs=4) as patt,       # att [128,256] bf16
            tc.tile_pool(name="pattl", bufs=4) as pattl,     # attT per j (live to end)
            tc.tile_pool(name="prcp", bufs=3) as prcp,       # recip [128,4] f32
            tc.tile_pool(name="pout", bufs=4) as pout,       # out staging f16
            tc.tile_pool(name="pconst", bufs=1) as pconst,   # small tiles
            tc.tile_pool(name="psc", bufs=2, space="PSUM") as psc,   # scores [128,1024]
            tc.tile_pool(name="pav", bufs=1, space="PSUM") as pav,   # av [128,512]
            tc.tile_pool(name="ptrp", bufs=1, space="PSUM") as ptrp, # transposes
            tc.tile_pool(name="pps", bufs=2, space="PSUM") as pps,   # general ring
        ):
            for _rep in range(reps):
                # ---- small constants (gpsimd queue, ahead of masks) ----
                # warm-up memset first: no deps, so the PE warm-up isn't
                # queued behind the const DMAs on DVE
                wrm = pconst.tile([128, 128], bf16, tag="wrm")
                nc.vector.memset(wrm[:], 0.0)

                # identity for PE transposes
                ident = pconst.tile([128, 128], bf16, tag="ident")
                make_identity(nc, ident[:])

                # ---- PE warm-up: junk matmuls ramp the p-state while DMAs
                # stream; their PSUM tile is write-only and recycled ----
                wps = pps.tile([128, 512], f32, tag="pp", name="warmps")
                for _w in range(48):
                    nc.tensor.matmul(wps[:, 0:128], wrm[:], wrm[:],
                                     start=True, stop=True)

                # ---- bulk loads (sync queue = HWDGE) ----
                wqb = pw.tile([128, 4 * 8 * 128], bf16, tag="w", name="wq")
                wq4 = wqb[:].rearrange("p (j c n) -> p j c n", j=4, c=8)
                wkb = pw.tile([128, 4 * 8 * 128], bf16, tag="w", name="wk")
                wk4 = wkb[:].rearrange("p (j c n) -> p j c n", j=4, c=8)
                xtb = pxt.tile([128, 8 * 1024], bf16, tag="xt", name="xt")
                xt3 = xtb[:].rearrange("p (c s) -> p c s", c=8)

                nc.sync.dma_start(out=wq4[:, 0], in_=wq[:, 0])
                nc.sync.dma_start(out=xt3[:, 0:4, 0:512], in_=xt[:, 0:4, 0:512])
                cst_t = pconst.tile([128, 12], f32, tag="cst", name="cst")
                nc.sync.dma_start(out=cst_t[:], in_=cst[:])
                scale_t = cst_t[:, 0:4]
                bq_t = cst_t[:, 4:8]
                bk_t = cst_t[:, 8:12]
                bqs_t = pconst.tile([128, 4], f32, tag="bqs")
                nc.vector.tensor_mul(bqs_t[:], bq_t, scale_t)
                # preload exp table during the DMA phase
                warm_t = pconst.tile([1, 4], f32, tag="warmexp")
                nc.scalar.activation(warm_t[:], scale_t[0:1, :], AF.Exp)
                nc.sync.dma_start(out=xt3[:, 4:8, 0:512], in_=xt[:, 4:8, 0:512])
                nc.sync.dma_start(out=wk4[:, 0], in_=wk[:, 0])
                nc.sync.dma_start(out=xt3[:, :, 512:1024],
                                  in_=xt[:, :, 512:1024])
                wvb = pw.tile([128, 8 * 512], bf16, tag="w", name="wv")
                wv3 = wvb[:].rearrange("p (c n) -> p c n", c=8)
                nc.sync.dma_start(out=wv3[:], in_=wv[:])
                for _j in (1, 2, 3):
                    nc.sync.dma_start(out=wq4[:, _j], in_=wq[:, _j])
                for _j in (1, 2, 3):
                    nc.sync.dma_start(out=wk4[:, _j], in_=wk[:, _j])
                wob = pw.tile([128, 4 * 1024], bf16, tag="w", name="wo")
                wo3 = wob[:].rearrange("p (r n) -> p r n", r=4)
                nc.sync.dma_start(out=wo3[:], in_=wo[:])

                # ---- mask pair loads (gpsimd queue), kt-granular ----
                mh = [None] * 4

                def load_mask(j):
                    t = pmask.tile([128, 8 * 2 * 1024], bf16, tag="mask",
                                   name=f"mp{j}")
                    tv = t[:].rearrange("p (k x) -> p k x", k=8)
                    for kt in range(8):
                        nc.gpsimd.dma_start(
                            out=tv[:, kt],
                            in_=mp[j, :, kt * 2048:(kt + 1) * 2048])
                    mh[j] = t

                load_mask(0)
                load_mask(1)

                # ---- Q^T/K^T projection pieces ----
                qtb = [None] * 4
                ktb = [None] * 4

                def qk_alloc(j):
                    qtb[j] = pqk.tile([128, 1024], bf16, tag="qk", name=f"qt{j}")
                    ktb[j] = pqk.tile([128, 1024], bf16, tag="qk", name=f"kt{j}")

                _qk_ps = {}

                # piece 0: Q s-half0, 1: K s-half0, 2: Q s-half1, 3: K s-half1
                def qk_half(j, piece, half, pool=None, ptag="pp", pshape=None):
                    is_q = piece % 2 == 0
                    sh = piece // 2
                    wsrc = wq4 if is_q else wk4
                    key = (j, piece)
                    if half == 0:
                        _qk_ps[key] = (pool or pps).tile(
                            pshape or [128, 512], f32, tag=ptag,
                            name=f"qkps{j}_{piece}")
                    psa = _qk_ps[key][:, 0:512]
                    for c8 in range(4 * half, 4 * half + 4):
                        nc.tensor.matmul(psa, wsrc[:, j, c8, :],
                                         xt3[:, c8, sh * 512:(sh + 1) * 512],
                                         start=(c8 == 0), stop=(c8 == 7))
                    if half == 1:
                        dst = (qtb[j] if is_q else ktb[j])[:, sh * 512:(sh + 1) * 512]
                        if is_q:
                            nc.vector.tensor_scalar(
                                dst, psa, cst_t[:, j:j + 1],
                                bqs_t[:, j:j + 1], ALU.mult, ALU.add)
                        else:
                            nc.vector.tensor_scalar_add(dst, psa,
                                                        cst_t[:, 8 + j:9 + j])

                def qk_piece(j, piece, pool=None, ptag="pp", pshape=None):
                    qk_half(j, piece, 0, pool, ptag, pshape)
                    qk_half(j, piece, 1, pool, ptag, pshape)

                # ---- V projection chunk st -> Vext [128 s, 8*65] ----
                vext = [None] * 8

                def vchunk(st, pool=None, ptag="pp", pshape=None):
                    vps = (pool or pps).tile(pshape or [128, 512], f32,
                                             tag=ptag, name=f"vps{st}")
                    vps = vps[:, 0:512]
                    for c8 in range(8):
                        nc.tensor.matmul(vps, xt3[:, c8, st * 128:(st + 1) * 128],
                                         wv3[:, c8, :], start=(c8 == 0), stop=(c8 == 7))
                    vt = pv.tile([128, 520], bf16, tag="vext", name=f"vext{st}")
                    v3 = vt[:].rearrange("p (h e) -> p h e", e=65)
                    # alternate eviction engine so neither ACT nor DVE piles up
                    if st % 2 == 0:
                        nc.scalar.activation(
                            v3[:, :, 0:64],
                            vps.rearrange("p (h e) -> p h e", e=64), AF.Copy)
                    else:
                        nc.vector.tensor_copy(
                            v3[:, :, 0:64],
                            vps.rearrange("p (h e) -> p h e", e=64))
                    nc.gpsimd.memset(v3[:, :, 64:65], 1.0)
                    vext[st] = vt

                # ---- attention machinery ----
                attl = [None] * 4
                pending_tr = []   # deferred (j, qc, att_tile, widx)

                def flush_transposes():
                    if not pending_tr:
                        return
                    j, qc, att, w = pending_tr.pop(0)
                    ptr_t = ptrp.tile([128, 1024], bf16, tag="ptr",
                                     name=f"ptr{w}")
                    for h in (0, 1):
                        for qt in (0, 1):
                            nc.tensor.transpose(
                                ptr_t[h * 64:(h + 1) * 64,
                                      qt * 128:(qt + 1) * 128],
                                att[:, (h * 2 + qt) * 64:(h * 2 + qt + 1) * 64],
                                ident[:])
                    if attl[j] is None:
                        attl[j] = pattl.tile([128, 1024], bf16, tag="attl",
                                             name=f"attl{j}")
                    nc.vector.tensor_copy(attl[j][:, qc * 256:(qc + 1) * 256],
                                          ptr_t[:, 0:256])

                # pending AV state: the avs of window w run one window later,
                # by which time every pt tile is ~a full window old, so they
                # never wait on the exp/mask chain.
                pend = {}   # pts, j, qc, widx, avt

                def pend_avs(kt):
                    # one kt's worth (4 matmuls) of the previous window's AV
                    if not pend:
                        return
                    if kt == 8:
                        # normalization: per-partition reciprocal+scalar mult
                        avt, pj, pqc, pw = (pend["avt"], pend["j"],
                                            pend["qc"], pend["widx"])
                        att = patt.tile([128, 256], bf16, tag="att",
                                        name=f"att{pw}")
                        rcp = prcp.tile([128, 4], f32, tag="rcp",
                                        name=f"rcp{pw}")
                        avv = avt[:, 0:260].rearrange("p (n e) -> p n e", e=65)
                        nc.vector.reciprocal(rcp[:], avv[:, :, 64])
                        # one broadcast multiply normalizes all 4 chains
                        nc.vector.tensor_mul(
                            att[:].rearrange("p (n e) -> p n e", e=64),
                            avv[:, :, 0:64],
                            rcp[:].to_broadcast([128, 4, 64]))
                        pending_tr.append((pj, pqc, att, pw))
                        pend.clear()
                        return
                    if kt == 0:
                        # single bank-sized av tile, one accumulation group:
                        # the first matmul's start lazily zeroes the whole 2KB
                        # zero region so all 4 chains accumulate from zero.
                        pend["avt"] = pav.tile([128, 512], f32, tag="av",
                                               name=f"av{pend['widx']}")
                    avt = pend["avt"]
                    ptk = pend["pts"][kt // 2]
                    par = kt % 2
                    pj = pend["j"]
                    for h in (0, 1):
                        hh = 2 * pj + h
                        for qt in (0, 1):
                            c = (h * 2 + qt) * 65
                            nc.tensor.matmul(
                                avt[:, c:c + 65],
                                ptk[:, h * 512 + par * 256 + qt * 128:
                                    h * 512 + par * 256 + (qt + 1) * 128],
                                vext[kt][:, hh * 65:(hh + 1) * 65],
                                start=(kt == 0 and h == 0 and qt == 0),
                                stop=(kt == 7 and h == 1 and qt == 1),
                                skip_group_check=True)

                # compressed dispatch: the deferred AVs' inputs are all ready
                # before the window starts, so run them in the first slots and
                # the norm at slot 4 -- 4 extra slots of margin for the
                # single-buffered av bank's WAR and the transpose chain.
                # Window 1 keeps the spread schedule (vext 6/7 land mid-window).
                PS_FAST = [[0, 1], [2, 3], [4, 5], [6, 7], [8], [], [], [], []]
                PS_SLOW = [[0], [1], [2], [3], [4], [5], [6], [7], [8]]

                def window(j, qc, widx, fillers):
                    qs = slice(qc * 256, (qc + 1) * 256)
                    psched = PS_SLOW if widx <= 1 else PS_FAST
                    # mask view permuted to (h, kt, q) to match pt layout
                    mpv = mh[j][:].rearrange("p (k h q) -> p h k q", k=8, h=2)
                    pts = [None] * 4
                    ps = None
                    for kt in range(9):
                        if kt < 8:
                            # kt-PAIRED scores: one [128,1024] tile per kt
                            # pair; A halves (tile_position (0,0)) in bank X
                            # (cols 0:512), B halves ((64,0)) in bank Y (cols
                            # 512:1024). Only the even kt starts the group --
                            # its lazy zero region covers the odd kt's cols.
                            par = kt % 2
                            if par == 0:
                                ps = psc.tile([128, 1024], f32, tag="sc",
                                              name=f"sc{widx}_{kt // 2}")
                            nc.tensor.matmul(
                                ps[:, par * 256:(par + 1) * 256],
                                ktb[j][0:64, kt * 128:(kt + 1) * 128],
                                qtb[j][0:64, qs],
                                start=(par == 0), stop=(par == 1),
                                tile_position=(0, 0), skip_group_check=True)
                            nc.tensor.matmul(
                                ps[:, 512 + par * 256:512 + (par + 1) * 256],
                                ktb[j][64:128, kt * 128:(kt + 1) * 128],
                                qtb[j][64:128, qs],
                                start=(par == 0), stop=(par == 1),
                                tile_position=(64, 0), skip_group_check=True)
                        for k in psched[kt]:
                            pend_avs(k)
                        for u in fillers[kt]:
                            u()
                        if kt < 8 and kt % 2 == 1:
                            # one exp + one mask multiply per kt PAIR
                            pt = ppt.tile([128, 1024], bf16, tag="pt",
                                          name=f"pt{widx}_{kt // 2}")
                            nc.scalar.activation(pt[:], ps[:], AF.Exp)
                            nc.vector.tensor_mul(
                                pt[:].rearrange("p (h k q) -> p h k q",
                                                h=2, k=2),
                                pt[:].rearrange("p (h k q) -> p h k q",
                                                h=2, k=2),
                                mpv[:, :, kt - 1:kt + 1, qs])
                            pts[kt // 2] = pt
                    pend.update(pts=pts, j=j, qc=qc, widx=widx)

                # ---- out projection chunk (qt, ch): PSUM -> f16 -> DRAM ----
                def outproj(qt, ch):
                    ops = pps.tile([128, 512], f32, tag="pp",
                                   name=f"ops{qt}_{ch}")
                    for rcx in range(4):
                        nc.tensor.matmul(ops[:],
                                         attl[rcx][:, qt * 128:(qt + 1) * 128],
                                         wo3[:, rcx, ch * 512:(ch + 1) * 512],
                                         start=(rcx == 0), stop=(rcx == 3))
                    ot = pout.tile([128, 512], f16, tag="out",
                                   name=f"ot{qt}_{ch}")
                    if ch == 0:
                        nc.vector.tensor_copy(ot[:], ops[:])
                    else:
                        nc.scalar.activation(ot[:], ops[:], AF.Copy)
                    nc.sync.dma_start(
                        out=out[qt * 128:(qt + 1) * 128, ch * 512:(ch + 1) * 512],
                        in_=ot[:])

                # ---- schedule ----
                # head chains spread over the (still idle) psc/pav banks so
                # the 2-deep pps ring doesn't serialize the prologue
                qk_alloc(0)
                qk_piece(0, 0)
                qk_piece(0, 1, psc, "sc", [128, 1024])
                qk_piece(0, 2)
                qk_piece(0, 3, pav, "av")
                vchunk(0)
                vchunk(1)

                _vps = {}

                def vA(st):
                    def g():
                        _vps[st] = pps.tile([128, 512], f32, tag="pp",
                                            name=f"vps{st}")
                        for c8 in range(4):
                            nc.tensor.matmul(
                                _vps[st][:], xt3[:, c8, st * 128:(st + 1) * 128],
                                wv3[:, c8, :], start=(c8 == 0), stop=False)
                    return g

                def vB(st):
                    def g():
                        vps = _vps[st]
                        for c8 in range(4, 8):
                            nc.tensor.matmul(
                                vps[:], xt3[:, c8, st * 128:(st + 1) * 128],
                                wv3[:, c8, :], start=False, stop=(c8 == 7))
                        vt = pv.tile([128, 520], bf16, tag="vext",
                                     name=f"vext{st}")
                        v3 = vt[:].rearrange("p (h e) -> p h e", e=65)
                        if st % 2 == 0:
                            nc.scalar.activation(
                                v3[:, :, 0:64],
                                vps[:].rearrange("p (h e) -> p h e", e=64),
                                AF.Copy)
                        else:
                            nc.vector.tensor_copy(
                                v3[:, :, 0:64],
                                vps[:].rearrange("p (h e) -> p h e", e=64))
                        nc.gpsimd.memset(v3[:, :, 64:65], 1.0)
                        vext[st] = vt
                    return g

                def qkh(j, piece, half):
                    def g():
                        if piece == 0 and half == 0:
                            qk_alloc(j)
                        qk_half(j, piece, half)
                    return g

                def op(qt, ch):
                    return lambda: outproj(qt, ch)

                T = flush_transposes

                def slots(*units_at):
                    # units_at: dict slot -> list of units
                    f = [[] for _ in range(9)]
                    for s, us in units_at:
                        f[s].extend(us)
                    return f

                # per-window filler plans. Constraints: vext[kt] ready before
                # window 1 slot kt (all vchunks inside window 0); Q/K sh0 of
                # pair j evicted before window (j,0), K sh1 before its slot 4,
                # Q sh1 before (j,2); out-proj chunks once their attl quarter
                # (evicted 2 windows after the producing window) exists.
                F = [
                    slots((0, [vA(2)]), (1, [vB(2)]), (2, [vA(3)]),
                          (3, [vB(3)]), (4, [vA(4)]), (5, [vB(4)]),
                          (6, [vA(5)]), (7, [vB(5)])),
                    slots((0, [T]), (1, [vA(6)]), (2, [vB(6)]),
                          (3, [vA(7)]), (4, [vB(7)]),
                          (5, [qkh(1, 0, 0)]), (7, [qkh(1, 0, 1)])),
                    slots((0, [T]), (1, [qkh(1, 1, 0)]), (4, [qkh(1, 1, 1)])),
                    slots((0, [T]), (1, [qkh(1, 3, 0)]), (4, [qkh(1, 3, 1)])),
                    slots((0, [T]), (1, [qkh(1, 2, 0)]), (4, [qkh(1, 2, 1)])),
                    slots((0, [T]), (1, [qkh(2, 0, 0)]), (4, [qkh(2, 0, 1)])),
                    slots((0, [T]), (1, [qkh(2, 1, 0)]), (4, [qkh(2, 1, 1)])),
                    slots((0, [T]), (1, [qkh(2, 3, 0)]), (4, [qkh(2, 3, 1)])),
                    slots((0, [T]), (1, [qkh(2, 2, 0)]), (4, [qkh(2, 2, 1)])),
                    slots((0, [T]), (1, [qkh(3, 0, 0)]), (4, [qkh(3, 0, 1)])),
                    slots((0, [T]), (1, [qkh(3, 1, 0)]), (4, [qkh(3, 1, 1)])),
                    slots((0, [T]), (1, [qkh(3, 3, 0)]), (4, [qkh(3, 3, 1)])),
                    slots((0, [T]), (1, [qkh(3, 2, 0)]), (4, [qkh(3, 2, 1)])),
                    slots((0, [T])),
                    slots((0, [T]), (1, [op(0, 0)]), (3, [op(0, 1)]),
                          (5, [op(1, 0)]), (7, [op(1, 1)])),
                    slots((0, [T]), (1, [op(2, 0)]), (3, [op(2, 1)]),
                          (5, [op(3, 0)]), (7, [op(3, 1)])),
                ]

                widx = 0
                for j in range(4):
                    for qc in range(4):
                        window(j, qc, widx, F[widx])
                        widx += 1
                        if (j, qc) == (0, 1):
                            load_mask(2)
                        elif (j, qc) == (1, 1):
                            load_mask(3)

                # tail: flush window 14's transposes (enables qt 4/5 chunks),
                # run the final window's AVs interleaved with those chunks,
                # then its norm + transposes and the last out-proj chunks.
                flush_transposes()   # att of window 14 -> attl[3] cols 512:768
                tail_ops = [op(4, 0), op(4, 1), op(5, 0), op(5, 1)]
                for kt in range(9):
                    pend_avs(kt)
                    if kt in (1, 3, 5, 7):
                        tail_ops.pop(0)()
                # open the last four chunks (rcx 0-2 need only attl[0..2])
                # before the final transpose flush; close after it
                t_ops = []
                for qt, ch in ((6, 0), (6, 1), (7, 0), (7, 1)):
                    ops = pps.tile([128, 512], f32, tag="pp",
                                   name=f"ops{qt}_{ch}")
                    for rcx in range(3):
                        nc.tensor.matmul(
                            ops[:], attl[rcx][:, qt * 128:(qt + 1) * 128],
                            wo3[:, rcx, ch * 512:(ch + 1) * 512],
                            start=(rcx == 0), stop=False)
                    t_ops.append((qt, ch, ops))
                    if (qt, ch) == (6, 1):
                        flush_transposes()   # att of w15 -> attl[3] cols 768:
                for qt, ch, ops in t_ops:
                    nc.tensor.matmul(ops[:],
                                     attl[3][:, qt * 128:(qt + 1) * 128],
                                     wo3[:, 3, ch * 512:(ch + 1) * 512],
                                     start=False, stop=True)
                    ot = pout.tile([128, 512], f16, tag="out",
                                   name=f"otf{qt}_{ch}")
                    if ch == 0:
                        nc.vector.tensor_copy(ot[:], ops[:])
                    else:
                        nc.scalar.activation(ot[:], ops[:], AF.Copy)
                    nc.sync.dma_start(
                        out=out[qt * 128:(qt + 1) * 128,
                                ch * 512:(ch + 1) * 512],
                        in_=ot[:])

    nc.compile()
    return nc


def _get_nc():
    global _NC_CACHE
    if _NC_CACHE is None:
        _NC_CACHE = _build()
    return _NC_CACHE


def _prep_inputs(x, Wq, bq, Wk, bk, Wv, bv, Wo, bo, temperature, sparse_mask):
    bfd = ml_dtypes.bfloat16
    x = np.asarray(x, np.float32)
    Wq = np.asarray(Wq, np.float32); Wk = np.asarray(Wk, np.float32)
    Wv = np.asarray(Wv, np.float32); Wo = np.asarray(Wo, np.float32)
    bq = np.asarray(bq, np.float32); bk = np.asarray(bk, np.float32)
    temp = np.asarray(temperature, np.float32).reshape(-1)
    mask = np.asarray(sparse_mask)

    in_maps = []
    for c in CORE_IDS:
        b, g = c // 2, c % 2
        cols = slice(g * LOC, (g + 1) * LOC)
        hs = slice(g * GH, (g + 1) * GH)
        xt_h = np.ascontiguousarray(
            x[b].T.reshape(8, 128, 1024).transpose(1, 0, 2)).astype(bfd)
        wq_h = np.ascontiguousarray(
            Wq[:, cols].reshape(8, 128, 4, 128).transpose(1, 2, 0, 3)).astype(bfd)
        wk_h = np.ascontiguousarray(
            Wk[:, cols].reshape(8, 128, 4, 128).transpose(1, 2, 0, 3)).astype(bfd)
        wv_h = np.ascontiguousarray(
            Wv[:, cols].reshape(8, 128, 512).transpose(1, 0, 2)).astype(bfd)
        wo_h = np.ascontiguousarray(
            Wo[cols, :].reshape(4, 128, 1024).transpose(1, 0, 2)).astype(bfd)
        # mask pairs: [4 j, 128 p, 8 kt, 2 h, 1024 q]; element (j,p,kt,h,q) =
        # sparse_mask[b, hs[2j+h], q, kt*128+p]
        mt = mask[b, hs].transpose(0, 2, 1)            # [8h, 1024k, 1024q]
        mp_h = np.ascontiguousarray(
            mt.reshape(4, 2, 8, 128, 1024).transpose(0, 3, 2, 1, 4)
        ).astype(bfd).reshape(4, 128, 16384)
        cst_h = np.stack([
            (np.repeat(temp[hs], D) / np.sqrt(D)).astype(np.float32),
            bq[cols].astype(np.float32),
            bk[cols].astype(np.float32),
        ]).reshape(3, 4, 128).transpose(2, 0, 1).reshape(128, 12)
        in_maps.append({
            "xt": xt_h, "wq": wq_h, "wk": wk_h, "wv": wv_h, "wo": wo_h,
            "mp": mp_h, "cst": np.ascontiguousarray(cst_h),
        })
    return in_maps


def kernel(**inputs):
    in_maps = _prep_inputs(**inputs)
    nc = _get_nc()
    res = run_bass_kernel_spmd(nc, in_maps, CORE_IDS)
    # unshard: row-parallel partial sum per batch + constant bias row
    # (softmax rows sum to 1 so bv contributes bv @ Wo to every row)
    bv = np.asarray(inputs["bv"], np.float32)
    bo = np.asarray(inputs["bo"], np.float32)
    Wo = np.asarray(inputs["Wo"], np.float32)
    brow = bv @ Wo + bo
    out = np.empty((B, S, HID), np.float32)
    for b in range(B):
        out[b] = (res.results[2 * b]["out"].astype(np.float32)
                  + res.results[2 * b + 1]["out"].astype(np.float32) + brow)
    return out
